# revision 1
# baseline (speedup 1.0000x reference)
"""Fused transformer block (pre-norm attn + MLP) for Trainium2, 8 cores.

Sharding: data-parallel over batch (32 batches -> 4 per core), no
collectives. Each core computes the full block on its shard.

Per-core dataflow (feature-major activations, tokens in the free dim):
  P1: LN1 stats via ones-matmul (f32r x + bf16 x^2), PE-broadcast of
      rs/mu*rs, DVE apply -> h1T [C, T] bf16; qkT = Wqk^T @ h1T
      (W-stationary); v = h1T^T @ Wv (h1-stationary, batch-aligned token
      tiles) stored token-major with an interleaved ones column per head
      (softmax denominators ride along attn@v as output row 64).
  P2 (per batch): 12 heads of scoresT (kT-stationary, keys-major), exp
      on ACT (no max subtraction: |scores| < 3 for these inputs), oU^T =
      v_ones-stationary @ expT, reciprocal + PE broadcast + DVE mult ->
      o_all; then proj for this batch (K=64 head-tile contraction) +
      residual vs restreamed x; x2 spilled to DRAM.
  P3: LN2 on restreamed x2 -> h2T.
  P4: FC1 + exact-erf Gelu (fused per-partition bias), FC2 + residual.

LN gains/biases and the attention scale are folded into the weights on
the host. Matmul operands bf16 (f32r for the LN sum inputs); fp32
matmuls run 4x slower on the PE. Psum and the residual stream fp32.
LN rsqrt via Ln+Exp so every ACT function before P4 shares one table
set (a table switch costs ~2.7us). SBUF pools are phase-scoped to fit
the 24MB SBUF; the x2 spill round-trips through DRAM.
"""
import numpy as np
import ml_dtypes
import concourse.bacc as bacc
import concourse.mybir as mybir
import concourse.tile as tile
from concourse.bass_utils import run_bass_kernel_spmd

F32 = mybir.dt.float32
F32R = mybir.dt.float32r
BF16 = mybir.dt.bfloat16
AF = mybir.ActivationFunctionType
ALU = mybir.AluOpType

B, N, C = 32, 577, 768
H, D = 12, 64
HID = 3072
NCORES = 8
BPC = B // NCORES            # 4 batches per core
T = BPC * N                  # 2308 tokens per core
CHUNKS = [(0, 512), (512, 512), (1024, 512), (1536, 512), (2048, 260)]
KTILES = [(0, 128), (128, 128), (256, 128), (384, 128), (512, 65)]
QCH = [(0, 512), (512, 65)]  # free-dim split of 577


def _ln_rows(nc, pool, ps_sum, ps_ssq, cw, c0, sfx):
    """Per-token LN stats from psum sums: returns (rs, murs) bf16 [1, cw]."""
    mu = pool.tile([1, cw], F32, name=f"mu{sfx}_{c0}", tag=f"mu{sfx}")
    nc.vector.tensor_scalar(mu[:], ps_sum[:], 1.0 / C, None, ALU.mult)
    t1 = pool.tile([1, cw], F32, name=f"t1{sfx}_{c0}", tag=f"t1{sfx}")
    nc.vector.tensor_tensor(t1[:], ps_sum[:], mu[:], ALU.mult)
    t2 = pool.tile([1, cw], F32, name=f"t2{sfx}_{c0}", tag=f"t2{sfx}")
    nc.vector.tensor_tensor(t2[:], ps_ssq[:], t1[:], ALU.subtract)
    t3 = pool.tile([1, cw], F32, name=f"t3{sfx}_{c0}", tag=f"t3{sfx}")
    nc.vector.tensor_scalar(t3[:], t2[:], 1.0 / C, 1e-5, ALU.mult, ALU.add)
    lnv = pool.tile([1, cw], F32, name=f"lnv{sfx}_{c0}", tag=f"lnv{sfx}")
    nc.scalar.activation(lnv[:], t3[:], AF.Ln)
    rs = pool.tile([1, cw], BF16, name=f"rs{sfx}_{c0}", tag=f"rs{sfx}")
    nc.scalar.activation(rs[:], lnv[:], AF.Exp, scale=-0.5)
    murs = pool.tile([1, cw], BF16, name=f"mr{sfx}_{c0}", tag=f"mr{sfx}")
    nc.vector.tensor_tensor(murs[:], mu[:], rs[:], ALU.mult)
    return rs, murs


def _build_nc():
    nc = bacc.Bacc("TRN2", target_bir_lowering=False, debug=False,
                   num_devices=NCORES)
    xT_d = nc.dram_tensor("xT", [C, T], F32R, kind="ExternalInput")
    wqkv_d = nc.dram_tensor("wqkv", [C, 3 * C], BF16, kind="ExternalInput")
    qkb_d = nc.dram_tensor("qkb", [128, 12], F32, kind="ExternalInput")
    vbb_d = nc.dram_tensor("vbb", [128, C], F32, kind="ExternalInput")
    wp_d = nc.dram_tensor("wp", [12, 64, C], BF16, kind="ExternalInput")
    pb_d = nc.dram_tensor("pb", [128, 6], F32, kind="ExternalInput")
    w1_d = nc.dram_tensor("w1", [C, HID], BF16, kind="ExternalInput")
    b1a_d = nc.dram_tensor("b1a", [128, 24], F32, kind="ExternalInput")
    w2_d = nc.dram_tensor("w2", [HID, C], BF16, kind="ExternalInput")
    b2a_d = nc.dram_tensor("b2a", [128, 6], F32, kind="ExternalInput")
    outT_d = nc.dram_tensor("outT", [C, T], F32, kind="ExternalOutput")

    with tile.TileContext(nc) as tc:
        with tc.tile_pool(name="cst", bufs=1) as cst, \
             tc.tile_pool(name="dram", bufs=1, space="DRAM") as drp:
            ones_bf = cst.tile([128, 1], BF16)
            nc.vector.memset(ones_bf[:], 1.0)
            ones_f = cst.tile([128, 1], F32)
            nc.vector.memset(ones_f[:], 1.0)
            ones_r = cst.tile([128, 1], F32R)
            nc.vector.tensor_copy(ones_r[:], ones_f[:])
            ones_row = cst.tile([1, 128], BF16)
            nc.vector.memset(ones_row[:], 1.0)
            qkb = cst.tile([128, 12], F32)
            nc.sync.dma_start(out=qkb[:], in_=qkb_d[:])
            vbb = cst.tile([128, C], F32)
            nc.sync.dma_start(out=vbb[:], in_=vbb_d[:])
            pb = cst.tile([128, 6], F32)
            nc.sync.dma_start(out=pb[:], in_=pb_d[:])
            b1a = cst.tile([128, 24], F32)
            nc.sync.dma_start(out=b1a[:], in_=b1a_d[:])
            b2a = cst.tile([128, 6], F32)
            nc.sync.dma_start(out=b2a[:], in_=b2a_d[:])
            x2s = drp.tile([C, T], F32, name="x2spill")

            with tc.tile_pool(name="qv", bufs=1) as qv, \
                 tc.tile_pool(name="vbp", bufs=4) as vbp:
                qkT = [qv.tile([128, T], BF16, name=f"qkT{n}") for n in range(12)]
                vbuf = {}
                for b in range(BPC):
                    for i in range(5):
                        vbuf[(b, i)] = vbp.tile([128, H * 65], BF16,
                                                name=f"vb{b}_{i}", tag=f"vb{i}")

                # ---------------- P1: LN1 + qkv + v ----------------
                with tc.tile_pool(name="p1w", bufs=1) as p1w:
                    wq = [p1w.tile([128, 3 * C], BF16, name=f"wq{k}")
                          for k in range(6)]
                    for k in range(6):
                        nc.sync.dma_start(
                            out=wq[k][:], in_=wqkv_d[k * 128:(k + 1) * 128, :])
                    h1 = [p1w.tile([128, T], BF16, name=f"h1_{k}")
                          for k in range(6)]
                    with tc.tile_pool(name="p1t", bufs=2) as p1t, \
                         tc.tile_pool(name="p1s", bufs=1) as p1s, \
                         tc.tile_pool(name="ps1", bufs=1, space="PSUM") as ps1, \
                         tc.tile_pool(name="psqk", bufs=2, space="PSUM") as psqk:
                        for (c0, cw) in CHUNKS:
                            xk = []
                            for k in range(6):
                                xt = p1s.tile([128, cw], F32R, name=f"x{k}_{c0}",
                                              tag=f"x{k}")
                                nc.sync.dma_start(
                                    out=xt[:],
                                    in_=xT_d[k * 128:(k + 1) * 128, c0:c0 + cw])
                                xk.append(xt)
                            ps_sum = ps1.tile([1, cw], F32, name=f"pss_{c0}",
                                              tag="ps_sum")
                            ps_ssq = ps1.tile([1, cw], F32, name=f"psq_{c0}",
                                              tag="ps_ssq")
                            for k in range(6):
                                xq = p1t.tile([128, cw], BF16, name=f"xq{k}_{c0}",
                                              tag=f"xq{k}")
                                nc.vector.tensor_tensor(xq[:], xk[k][:], xk[k][:],
                                                        ALU.mult)
                                nc.tensor.matmul(ps_sum[:], ones_r[:], xk[k][:],
                                                 start=(k == 0), stop=(k == 5))
                                nc.tensor.matmul(ps_ssq[:], ones_bf[:], xq[:],
                                                 start=(k == 0), stop=(k == 5))
                            rs, murs = _ln_rows(nc, p1s, ps_sum, ps_ssq,
                                                cw, c0, "")
                            ps_rs = p1t.tile([128, cw], BF16, name=f"prs_{c0}",
                                             tag="ps_rs")
                            nc.gpsimd.partition_broadcast(ps_rs[:], rs[:])
                            ps_mu = p1t.tile([128, cw], BF16, name=f"pmu_{c0}",
                                             tag="ps_mu")
                            nc.gpsimd.partition_broadcast(ps_mu[:], murs[:])
                            for k in range(6):
                                tmp = p1s.tile([128, cw], F32, name=f"tp{k}_{c0}",
                                               tag=f"tp{k}")
                                nc.vector.tensor_tensor(tmp[:], xk[k][:],
                                                        ps_rs[:], ALU.mult)
                                nc.vector.tensor_tensor(h1[k][:, c0:c0 + cw],
                                                        tmp[:], ps_mu[:],
                                                        ALU.subtract)
                            for n in range(12):
                                pq = psqk.tile([128, cw], F32, name=f"pq{n}_{c0}",
                                               tag="psqk")
                                for k in range(6):
                                    nc.tensor.matmul(
                                        pq[:], wq[k][:, n * 128:(n + 1) * 128],
                                        h1[k][:, c0:c0 + cw],
                                        start=(k == 0), stop=(k == 5))
                                nc.scalar.activation(qkT[n][:, c0:c0 + cw],
                                                     pq[:], AF.Identity,
                                                     bias=qkb[:, n:n + 1])
                    # v matmuls: batch-aligned token tiles, h1-stationary
                    with tc.tile_pool(name="psv", bufs=2, space="PSUM") as psv:
                        for b in range(BPC):
                            for i, (kt0, kr) in enumerate(KTILES):
                                m0 = b * N + kt0
                                pv = psv.tile([kr, C], F32, name=f"pv{b}_{i}",
                                              tag="psv")
                                for k in range(6):
                                    nc.tensor.matmul(
                                        pv[:, 0:512], h1[k][:, m0:m0 + kr],
                                        wq[k][:, 1536:2048],
                                        start=(k == 0), stop=(k == 5))
                                    nc.tensor.matmul(
                                        pv[:, 512:768], h1[k][:, m0:m0 + kr],
                                        wq[k][:, 2048:2304],
                                        start=(k == 0), stop=(k == 5))
                                vm = vbuf[(b, i)]
                                ones_ap = vm[:kr].rearrange(
                                    "p (h e) -> p h e", e=65)[:, :, 64]
                                nc.vector.memset(ones_ap, 1.0)
                                for h in range(H):
                                    nc.vector.tensor_tensor(
                                        vm[:kr, h * 65:h * 65 + 64],
                                        pv[:kr, h * 64:(h + 1) * 64],
                                        vbb[:kr, h * 64:(h + 1) * 64], ALU.add)

                # ------- P2: attention + per-batch proj/residual -------
                # Heads are software-pipelined: scores/exp for head i+1 are
                # emitted before the attn@v/normalize tail of head i, so the
                # PE never drains while ACT runs exp (keeps HAM at 2.4GHz).
                with tc.tile_pool(name="wpp", bufs=1) as wpp, \
                     tc.tile_pool(name="oal", bufs=2) as oal, \
                     tc.tile_pool(name="p2t", bufs=3) as p2t, \
                     tc.tile_pool(name="pexp", bufs=12) as pexp, \
                     tc.tile_pool(name="p2pr", bufs=1) as p2pr, \
                     tc.tile_pool(name="pss", bufs=2, space="PSUM") as pss, \
                     tc.tile_pool(name="pso", bufs=1, space="PSUM") as pso, \
                     tc.tile_pool(name="psp", bufs=2, space="PSUM") as psp:
                    wp_sb = [wpp.tile([64, C], BF16, name=f"wp{i}")
                             for i in range(12)]
                    for i in range(12):
                        nc.sync.dma_start(out=wp_sb[i][:], in_=wp_d[i])

                    def emit_head(b, h):
                        base = b * N
                        p0 = (h % 2) * 64
                        q_sl = qkT[h // 2][p0:p0 + 64, base:base + N]
                        k_sl = qkT[6 + h // 2][p0:p0 + 64, base:base + N]
                        exps = []
                        for i, (kt0, kr) in enumerate(KTILES):
                            ps_s = pss.tile([kr, N], F32,
                                            name=f"ss{b}_{h}_{i}", tag="ps_s")
                            for (qc0, qcw) in QCH:
                                nc.tensor.matmul(
                                    ps_s[:, qc0:qc0 + qcw],
                                    k_sl[:, kt0:kt0 + kr],
                                    q_sl[:, qc0:qc0 + qcw],
                                    start=True, stop=True)
                            e = pexp.tile([kr, N], BF16,
                                          name=f"e{b}_{h}_{i}", tag="exp")
                            nc.scalar.activation(e[:], ps_s[:], AF.Exp)
                            exps.append((e, kr))
                        return exps

                    def emit_tail(b, h, exps, o_b):
                        ps_o = pso.tile([65, N], F32, name=f"po{b}_{h}",
                                        tag="ps_o")
                        for (qc0, qcw) in QCH:
                            for i, (e, kr) in enumerate(exps):
                                nc.tensor.matmul(
                                    ps_o[:, qc0:qc0 + qcw],
                                    vbuf[(b, i)][:kr, h * 65:(h + 1) * 65],
                                    e[:kr, qc0:qc0 + qcw],
                                    start=(i == 0), stop=(i == 4))
                        oU = p2t.tile([65, N], F32, name=f"oU{b}_{h}",
                                      tag="oU")
                        nc.vector.tensor_copy(oU[:], ps_o[:])
                        rec = p2t.tile([1, N], BF16, name=f"rc{b}_{h}",
                                       tag="rec")
                        with nc.allow_low_precision(reason="softmax denom"):
                            nc.vector.reciprocal(rec[:], oU[64:65, :])
                        bc = p2t.tile([64, N], BF16, name=f"bc{b}_{h}",
                                      tag="bc")
                        nc.gpsimd.partition_broadcast(bc[:], rec[:])
                        nc.vector.tensor_tensor(o_b[h][:], oU[0:64, :],
                                                bc[:], ALU.mult)

                    def emit_proj(b, o_b):
                        base = b * N
                        for (qc0, qcw) in QCH:
                            for n in range(6):
                                pp = psp.tile([128, qcw], F32,
                                              name=f"pp{b}_{n}_{qc0}",
                                              tag="psp")
                                for kh in range(12):
                                    nc.tensor.matmul(
                                        pp[:],
                                        wp_sb[kh][:, n * 128:(n + 1) * 128],
                                        o_b[kh][:, qc0:qc0 + qcw],
                                        start=(kh == 0), stop=(kh == 11))
                                tp = p2pr.tile([128, qcw], F32,
                                               name=f"tpp{b}_{n}_{qc0}",
                                               tag=f"tpp{n}")
                                nc.vector.tensor_scalar(
                                    tp[:], pp[:], pb[:, n:n + 1], None, ALU.add)
                                xr = p2pr.tile([128, qcw], F32R,
                                               name=f"xr{b}_{n}_{qc0}",
                                               tag=f"xr{n}")
                                nc.sync.dma_start(
                                    out=xr[:],
                                    in_=xT_d[n * 128:(n + 1) * 128,
                                             base + qc0:base + qc0 + qcw])
                                x2 = p2pr.tile([128, qcw], F32,
                                               name=f"x2_{b}_{n}_{qc0}",
                                               tag=f"x2_{n}")
                                nc.vector.tensor_tensor(x2[:], tp[:], xr[:],
                                                        ALU.add)
                                nc.sync.dma_start(
                                    out=x2s[n * 128:(n + 1) * 128,
                                            base + qc0:base + qc0 + qcw],
                                    in_=x2[:])

                    o_tiles = {}
                    pending = None
                    for b in range(BPC):
                        o_tiles[b] = [oal.tile([64, N], BF16,
                                               name=f"oa{b}_{h}", tag=f"oa{h}")
                                      for h in range(H)]
                        for h in range(H):
                            exps = emit_head(b, h)
                            if pending is not None:
                                pb_, ph_, pe_ = pending
                                emit_tail(pb_, ph_, pe_, o_tiles[pb_])
                                if ph_ == H - 1:
                                    emit_proj(pb_, o_tiles[pb_])
                            pending = (b, h, exps)
                    pb_, ph_, pe_ = pending
                    emit_tail(pb_, ph_, pe_, o_tiles[pb_])
                    emit_proj(pb_, o_tiles[pb_])

            # ---------------- P3: LN2 -> h2 ----------------
            with tc.tile_pool(name="h2p", bufs=1) as h2p:
                h2 = [h2p.tile([128, T], BF16, name=f"h2_{k}") for k in range(6)]
                with tc.tile_pool(name="p3b", bufs=2) as p3b, \
                     tc.tile_pool(name="p3s", bufs=1) as p3s, \
                     tc.tile_pool(name="ps3", bufs=1, space="PSUM") as ps3:
                    for (c0, cw) in CHUNKS:
                        xk2 = []
                        ps_sum = ps3.tile([1, cw], F32, name=f"p2s_{c0}",
                                          tag="ps_sum2")
                        ps_ssq = ps3.tile([1, cw], F32, name=f"p2q_{c0}",
                                          tag="ps_ssq2")
                        for k in range(6):
                            xr2 = p3b.tile([128, cw], F32R, name=f"y{k}_{c0}",
                                           tag=f"y{k}")
                            nc.sync.dma_start(
                                out=xr2[:],
                                in_=x2s[k * 128:(k + 1) * 128,
                                        c0:c0 + cw].bitcast(F32R))
                            xk2.append(xr2)
                            xq2 = p3b.tile([128, cw], BF16, name=f"yq{k}_{c0}",
                                           tag=f"yq{k}")
                            nc.vector.tensor_tensor(xq2[:], xr2[:], xr2[:],
                                                    ALU.mult)
                            nc.tensor.matmul(ps_sum[:], ones_r[:], xr2[:],
                                             start=(k == 0), stop=(k == 5))
                            nc.tensor.matmul(ps_ssq[:], ones_bf[:], xq2[:],
                                             start=(k == 0), stop=(k == 5))
                        rs, murs = _ln_rows(nc, p3s, ps_sum, ps_ssq, cw, c0, "2")
                        ps_rs = p3b.tile([128, cw], BF16, name=f"pr2_{c0}",
                                         tag="ps_rs2")
                        nc.gpsimd.partition_broadcast(ps_rs[:], rs[:])
                        ps_mu = p3b.tile([128, cw], BF16, name=f"pm2_{c0}",
                                         tag="ps_mu2")
                        nc.gpsimd.partition_broadcast(ps_mu[:], murs[:])
                        for k in range(6):
                            tmp = p3s.tile([128, cw], F32, name=f"tq{k}_{c0}",
                                           tag=f"tq{k}")
                            nc.vector.tensor_tensor(tmp[:], xk2[k][:],
                                                    ps_rs[:], ALU.mult)
                            nc.vector.tensor_tensor(h2[k][:, c0:c0 + cw],
                                                    tmp[:], ps_mu[:],
                                                    ALU.subtract)

                # ---------------- P4: MLP ----------------
                with tc.tile_pool(name="w12", bufs=1) as w12, \
                     tc.tile_pool(name="p4t", bufs=2) as p4t, \
                     tc.tile_pool(name="pgl", bufs=1) as pgl, \
                     tc.tile_pool(name="ps41", bufs=3, space="PSUM") as ps41, \
                     tc.tile_pool(name="ps42", bufs=2, space="PSUM") as ps42:
                    w1_sb = [w12.tile([128, HID], BF16, name=f"w1_{k}")
                             for k in range(6)]
                    for k in range(6):
                        nc.sync.dma_start(out=w1_sb[k][:],
                                          in_=w1_d[k * 128:(k + 1) * 128, :])
                    w2_sb = [w12.tile([128, C], BF16, name=f"w2_{k}")
                             for k in range(24)]
                    for k in range(24):
                        nc.sync.dma_start(out=w2_sb[k][:],
                                          in_=w2_d[k * 128:(k + 1) * 128, :])
                    for (c0, cw) in CHUNKS:
                        gl = []
                        for n1 in range(24):
                            p1p = ps41.tile([128, cw], F32,
                                            name=f"p41_{n1}_{c0}", tag="ps41")
                            for k in range(6):
                                nc.tensor.matmul(
                                    p1p[:],
                                    w1_sb[k][:, n1 * 128:(n1 + 1) * 128],
                                    h2[k][:, c0:c0 + cw],
                                    start=(k == 0), stop=(k == 5))
                            g = pgl.tile([128, cw], BF16, name=f"gl{n1}_{c0}",
                                         tag=f"gl{n1}")
                            nc.scalar.activation(g[:], p1p[:], AF.Gelu,
                                                 bias=b1a[:, n1:n1 + 1])
                            gl.append(g)
                        for n2 in range(6):
                            p2p = ps42.tile([128, cw], F32,
                                            name=f"p42_{n2}_{c0}", tag="ps42")
                            for k2 in range(24):
                                nc.tensor.matmul(
                                    p2p[:],
                                    w2_sb[k2][:, n2 * 128:(n2 + 1) * 128],
                                    gl[k2][:],
                                    start=(k2 == 0), stop=(k2 == 23))
                            t2o = p4t.tile([128, cw], F32,
                                           name=f"t2o{n2}_{c0}", tag="t2o")
                            nc.scalar.activation(t2o[:], p2p[:], AF.Identity,
                                                 bias=b2a[:, n2:n2 + 1])
                            xr2 = p4t.tile([128, cw], F32,
                                           name=f"x2r{n2}_{c0}",
                                           tag=f"x2r{n2}")
                            nc.sync.dma_start(
                                out=xr2[:],
                                in_=x2s[n2 * 128:(n2 + 1) * 128, c0:c0 + cw])
                            oo = p4t.tile([128, cw], F32, name=f"oo{n2}_{c0}",
                                          tag="oo")
                            nc.vector.tensor_tensor(oo[:], t2o[:], xr2[:],
                                                    ALU.add)
                            nc.sync.dma_start(
                                out=outT_d[n2 * 128:(n2 + 1) * 128,
                                           c0:c0 + cw],
                                in_=oo[:])
    nc.compile()
    return nc


_CACHE = {}


def _prep_shared(inputs):
    f32 = np.float32
    qkv_w = np.asarray(inputs["qkv_w"], f32)
    ln1_g = np.asarray(inputs["ln1_g"], f32)
    ln1_b = np.asarray(inputs["ln1_b"], f32)
    qkv_b = np.asarray(inputs["qkv_b"], f32)
    W = qkv_w * ln1_g[:, None]
    bq = ln1_b @ qkv_w + qkv_b
    W = W.copy()
    W[:, :C] *= 0.125
    bq = bq.copy()
    bq[:C] *= 0.125

    proj_w = np.asarray(inputs["proj_w"], f32)
    fc1_w = np.asarray(inputs["fc1_w"], f32)
    ln2_g = np.asarray(inputs["ln2_g"], f32)
    ln2_b = np.asarray(inputs["ln2_b"], f32)
    fc1_b = np.asarray(inputs["fc1_b"], f32)
    W1 = fc1_w * ln2_g[:, None]
    b1 = ln2_b @ fc1_w + fc1_b
    fc2_w = np.asarray(inputs["fc2_w"], f32)

    bf = ml_dtypes.bfloat16
    return {
        "wqkv": np.ascontiguousarray(W.astype(bf)),
        "qkb": np.ascontiguousarray(bq[:2 * C].reshape(12, 128).T.astype(f32)),
        "vbb": np.ascontiguousarray(np.tile(bq[2 * C:], (128, 1)).astype(f32)),
        "wp": np.ascontiguousarray(proj_w.reshape(12, 64, C).astype(bf)),
        "pb": np.ascontiguousarray(
            np.asarray(inputs["proj_b"], f32).reshape(6, 128).T),
        "w1": np.ascontiguousarray(W1.astype(bf)),
        "b1a": np.ascontiguousarray(b1.reshape(24, 128).T.astype(f32)),
        "w2": np.ascontiguousarray(fc2_w.astype(bf)),
        "b2a": np.ascontiguousarray(
            np.asarray(inputs["fc2_b"], f32).reshape(6, 128).T),
    }


def kernel(**inputs):
    if "nc" not in _CACHE:
        _CACHE["nc"] = _build_nc()
    nc = _CACHE["nc"]
    x = np.asarray(inputs["x"], np.float32)
    shared = _prep_shared(inputs)
    in_maps = []
    for c in range(NCORES):
        xT = np.ascontiguousarray(
            x[c * BPC:(c + 1) * BPC].reshape(T, C).T)
        m = {"xT": xT}
        m.update(shared)
        in_maps.append(m)
    res = run_bass_kernel_spmd(nc, in_maps, list(range(NCORES)))
    out = np.empty((B, N, C), np.float32)
    for c in range(NCORES):
        outT = res.results[c]["outT"]
        out[c * BPC:(c + 1) * BPC] = outT.T.reshape(BPC, N, C)
    return out



# revision 19
# speedup vs baseline: 1.3146x; 1.3146x over previous
"""Fused transformer block (pre-norm attn + MLP) for Trainium2, 8 cores.

Sharding: data-parallel over batch (32 batches -> 4 per core), no
collectives. Each core computes the full block on its shard.

v2 design notes (vs v1 baseline at 1051us):
- Every matmul runs in the PE's 128x128 tile mode (K=128 contraction or
  round-up): scores use a zero-padded q buffer so the K=64 head_dim
  contraction becomes K=128 with junk-times-zero rows; LN stats use an
  all-ones [128,128] stationary so the column sums come out broadcast
  across all 128 psum partitions (no 1-row matmuls, no gpsimd
  partition-broadcast for LN). Mixed tile modes force PE drains and kept
  the HAM clock gate at K=4/8 (1.2GHz) for the entire attention phase in
  v1 (440us window at half clock).
- x streams in bf16 (host-cast); LN apply reads bf16 x directly.
- proj contracts head PAIRS (o stacked [128,N]) -> K=128, half the
  matmuls of per-head K=64.
- softmax denominators ride attn@v as psum row 64 (ones column in v);
  all 12 heads' denominators are copied into one [12,N] tile and
  reciprocal'd in ONE DVE instruction (v1 spent 45us/batch on [1,N]
  reciprocals).
- x2 (attn residual) stays SBUF-resident in bf16; LN2 stats/apply are
  pipelined per chunk inside the MLP phase (v1 had a 90us P3 phase with
  an idle PE and a DRAM round trip).
- Phases software-pipeline: LN stats of chunk c+1 are emitted before the
  qkv matmuls of chunk c; scores of head h+1 before attn@v of head h;
  the first scores of batch b+1 before proj of batch b; MLP stats of
  chunk c+1 between FC1(c) and FC2(c).
LN gains/biases and the attention scale are folded into the weights on
the host. All matmul operands bf16; psum f32. ACT exp has no max
subtraction (|scores| < 3). Ln/Exp share one ACT table set; Gelu only
appears in the MLP phase (one table switch).
"""
import numpy as np
import ml_dtypes
import concourse.bacc as bacc
import concourse.mybir as mybir
import concourse.tile as tile
from concourse.bass_utils import run_bass_kernel_spmd

F32 = mybir.dt.float32
BF16 = mybir.dt.bfloat16
AF = mybir.ActivationFunctionType
ALU = mybir.AluOpType

B, N, C = 32, 577, 768
H, D = 12, 64
HID = 3072
NCORES = 8
BPC = B // NCORES            # 4 batches per core
T = BPC * N                  # 2308 tokens per core
CHUNKS = [(0, 512), (512, 512), (1024, 512), (1536, 512), (2048, 260)]
# P1 chunks are batch-aligned so every v-evac psum->vbuf copy starts at
# partition 0 (DVE ops with >64 channels must be quadrant-aligned).
P1CH = [(b * 577 + o, w) for b in range(4) for (o, w) in ((0, 512), (512, 65))]
KTILES = [(0, 128), (128, 128), (256, 128), (384, 128), (512, 65)]
QCH = [(0, 512), (512, 65)]  # free-dim split of 577 (psum bank = 512 f32)


def _build_nc():
    nc = bacc.Bacc("TRN2", target_bir_lowering=False, debug=False,
                   num_devices=NCORES)
    xT_d = nc.dram_tensor("xT", [C, T], BF16, kind="ExternalInput")
    wqkv_d = nc.dram_tensor("wqkv", [C, 3 * C], BF16, kind="ExternalInput")
    qkb_d = nc.dram_tensor("qkb", [128, 12], F32, kind="ExternalInput")
    vbb_d = nc.dram_tensor("vbb", [128, C], F32, kind="ExternalInput")
    wp_d = nc.dram_tensor("wp", [6, 128, C], BF16, kind="ExternalInput")
    pb_d = nc.dram_tensor("pb", [128, 6], F32, kind="ExternalInput")
    w1_d = nc.dram_tensor("w1", [C, HID], BF16, kind="ExternalInput")
    b1a_d = nc.dram_tensor("b1a", [128, 24], F32, kind="ExternalInput")
    w2_d = nc.dram_tensor("w2", [HID, C], BF16, kind="ExternalInput")
    b2a_d = nc.dram_tensor("b2a", [128, 6], F32, kind="ExternalInput")
    outT_d = nc.dram_tensor("outT", [C, T], F32, kind="ExternalOutput")

    with tile.TileContext(nc) as tc:
        with tc.tile_pool(name="cst", bufs=1) as cst, \
             tc.tile_pool(name="x2p", bufs=1) as x2p:
            ones128 = cst.tile([128, 128], BF16)
            nc.vector.memset(ones128[:], 1.0)
            qkb = cst.tile([128, 12], F32)
            nc.sync.dma_start(out=qkb[:], in_=qkb_d[:])
            vbb = cst.tile([128, C], F32)
            nc.sync.dma_start(out=vbb[:], in_=vbb_d[:])
            pb = cst.tile([128, 6], F32)
            nc.sync.dma_start(out=pb[:], in_=pb_d[:])
            b1a = cst.tile([128, 24], F32)
            nc.sync.dma_start(out=b1a[:], in_=b1a_d[:])
            b2a = cst.tile([128, 6], F32)
            nc.sync.dma_start(out=b2a[:], in_=b2a_d[:])
            x2 = [x2p.tile([128, T], BF16, name=f"x2_{k}") for k in range(6)]

            with tc.tile_pool(name="qks", bufs=1) as qks, \
                 tc.tile_pool(name="vbp", bufs=1) as vbp:
                qk_sb = [qks.tile([128, T], BF16, name=f"qk{n}")
                         for n in range(12)]
                vbuf = {}
                for b in range(BPC):
                    for i in range(5):
                        vbuf[(b, i)] = vbp.tile([128, H * 65], BF16,
                                                name=f"vb{b}_{i}")
                        ocol = vbuf[(b, i)].rearrange(
                            "p (h e) -> p h e", e=65)[:, :, 64]
                        nc.vector.memset(ocol, 1.0)

                # ---------------- P1: LN1 + qk + v ----------------
                with tc.tile_pool(name="p1w", bufs=1) as p1w:
                    wq = [p1w.tile([128, 3 * C], BF16, name=f"wq{k}")
                          for k in range(6)]
                    for k in range(6):
                        nc.sync.dma_start(
                            out=wq[k][:], in_=wqkv_d[k * 128:(k + 1) * 128, :])
                    with tc.tile_pool(name="p1x", bufs=2) as p1x, \
                         tc.tile_pool(name="p1h", bufs=2) as p1h, \
                         tc.tile_pool(name="p1s", bufs=2) as p1s, \
                         tc.tile_pool(name="ps1", bufs=2, space="PSUM") as ps1, \
                         tc.tile_pool(name="psqk", bufs=2, space="PSUM") as psqk, \
                         tc.tile_pool(name="psv", bufs=1, space="PSUM") as psv:
                        xc = {}
                        h1c = {}

                        def emit_stats(ci):
                            c0, cw = P1CH[ci]
                            xs = []
                            for k in range(6):
                                xt = p1x.tile([128, cw], BF16,
                                              name=f"x{k}_{c0}", tag=f"x{k}")
                                nc.sync.dma_start(
                                    out=xt[:],
                                    in_=xT_d[k * 128:(k + 1) * 128,
                                             c0:c0 + cw])
                                xs.append(xt)
                            xc[ci] = xs
                            ps_sum = ps1.tile([128, cw], F32,
                                              name=f"pss_{c0}", tag="ps_sum")
                            ps_ssq = ps1.tile([128, cw], F32,
                                              name=f"psq_{c0}", tag="ps_ssq")
                            for k in range(6):
                                xq = p1x.tile([128, cw], BF16,
                                              name=f"xq{k}_{c0}", tag=f"xq{k}")
                                nc.vector.tensor_tensor(xq[:], xs[k][:],
                                                        xs[k][:], ALU.mult)
                                nc.tensor.matmul(ps_sum[:], ones128[:],
                                                 xs[k][:],
                                                 start=(k == 0), stop=(k == 5))
                                nc.tensor.matmul(ps_ssq[:], ones128[:], xq[:],
                                                 start=(k == 0), stop=(k == 5))
                            return ps_sum, ps_ssq

                        def emit_ln(ci, ps_sum, ps_ssq):
                            c0, cw = P1CH[ci]
                            mu = p1s.tile([128, cw], F32, name=f"mu_{c0}",
                                          tag="mu")
                            nc.vector.tensor_scalar(mu[:], ps_sum[:], 1.0 / C,
                                                    None, ALU.mult)
                            t1 = p1s.tile([128, cw], F32, name=f"t1_{c0}",
                                          tag="tt")
                            nc.vector.tensor_tensor(t1[:], ps_sum[:], mu[:],
                                                    ALU.mult)
                            t2 = p1s.tile([128, cw], F32, name=f"t2_{c0}",
                                          tag="tt")
                            nc.vector.tensor_tensor(t2[:], ps_ssq[:], t1[:],
                                                    ALU.subtract)
                            t3 = p1s.tile([128, cw], F32, name=f"t3_{c0}",
                                          tag="tt")
                            nc.vector.tensor_scalar(t3[:], t2[:], 1.0 / C,
                                                    1e-5, ALU.mult, ALU.add)
                            lnv = p1s.tile([128, cw], F32, name=f"lnv_{c0}",
                                           tag="tt")
                            nc.scalar.activation(lnv[:], t3[:], AF.Ln)
                            rs = p1s.tile([128, cw], BF16, name=f"rs_{c0}",
                                          tag="rs")
                            nc.scalar.activation(rs[:], lnv[:], AF.Exp,
                                                 scale=-0.5)
                            murs = p1s.tile([128, cw], BF16, name=f"mr_{c0}",
                                            tag="mr")
                            nc.vector.tensor_tensor(murs[:], mu[:], rs[:],
                                                    ALU.mult)
                            hs = []
                            for k in range(6):
                                tmp = p1s.tile([128, cw], F32,
                                               name=f"tp{k}_{c0}", tag="tp")
                                nc.vector.tensor_tensor(tmp[:], xc[ci][k][:],
                                                        rs[:], ALU.mult)
                                ht = p1h.tile([128, cw], BF16,
                                              name=f"h{k}_{c0}", tag=f"h{k}")
                                nc.vector.tensor_tensor(ht[:], tmp[:], murs[:],
                                                        ALU.subtract)
                                hs.append(ht)
                            h1c[ci] = hs

                        def emit_qkv(ci):
                            c0, cw = P1CH[ci]
                            hs = h1c[ci]
                            for n in range(12):
                                pq = psqk.tile([128, cw], F32,
                                               name=f"pq{n}_{c0}", tag="psqk")
                                for k in range(6):
                                    nc.tensor.matmul(
                                        pq[:], wq[k][:, n * 128:(n + 1) * 128],
                                        hs[k][:], start=(k == 0), stop=(k == 5))
                                if n < 6:
                                    nc.scalar.activation(
                                        qk_sb[n][:, c0:c0 + cw], pq[:],
                                        AF.Identity, bias=qkb[:, n:n + 1])
                                else:
                                    nc.vector.tensor_scalar(
                                        qk_sb[n][:, c0:c0 + cw], pq[:],
                                        qkb[:, n:n + 1], None, ALU.add)
                            # v for this chunk (chunks are batch-aligned,
                            # so each KTILE lies fully inside one chunk and
                            # all partition bases are 0)
                            b = c0 // N
                            o0 = c0 - b * N
                            for i, (kt0, kr) in enumerate(KTILES):
                                if kt0 < o0 or kt0 >= o0 + cw:
                                    continue
                                lt = kt0 - o0
                                pv = psv.tile([128, C], F32,
                                              name=f"pv_{c0}_{lt}", tag="psv")
                                for k in range(6):
                                    nc.tensor.matmul(
                                        pv[:kr, 0:512],
                                        hs[k][:, lt:lt + kr],
                                        wq[k][:, 1536:2048],
                                        start=(k == 0), stop=(k == 5))
                                for k in range(6):
                                    nc.tensor.matmul(
                                        pv[:kr, 512:768],
                                        hs[k][:, lt:lt + kr],
                                        wq[k][:, 2048:2304],
                                        start=(k == 0), stop=(k == 5))
                                vm = vbuf[(b, i)]
                                dst = vm[0:kr].rearrange(
                                    "p (h e) -> p h e", e=65)[:, :, 0:64]
                                srcv = pv[0:kr].rearrange(
                                    "p (h e) -> p h e", e=64)
                                bia = vbb[0:kr].rearrange(
                                    "p (h e) -> p h e", e=64)
                                nc.vector.tensor_tensor(dst, srcv, bia,
                                                        ALU.add)
                            del h1c[ci], xc[ci]

                        emit_ln(0, *emit_stats(0))
                        emit_ln(1, *emit_stats(1))
                        for ci in range(len(P1CH)):
                            emit_qkv(ci)
                            if ci + 2 < len(P1CH):
                                emit_ln(ci + 2, *emit_stats(ci + 2))

                # ---------------- P2: attention + proj ----------------
                with tc.tile_pool(name="wpp", bufs=1) as wpp, \
                     tc.tile_pool(name="qbp", bufs=1) as qbp, \
                     tc.tile_pool(name="oal", bufs=2) as oal, \
                     tc.tile_pool(name="p2t", bufs=2) as p2t, \
                     tc.tile_pool(name="pexp", bufs=2) as pexp, \
                     tc.tile_pool(name="pss", bufs=2, space="PSUM") as pss, \
                     tc.tile_pool(name="pso", bufs=2, space="PSUM") as pso:
                    wp_sb = [wpp.tile([128, C], BF16, name=f"wp{j}")
                             for j in range(6)]
                    for j in range(6):
                        nc.sync.dma_start(out=wp_sb[j][:], in_=wp_d[j])
                    qb = [qbp.tile([128, N], BF16, name=f"qb{par}")
                          for par in range(2)]
                    nc.vector.memset(qb[0][64:128, :], 0.0)
                    nc.vector.memset(qb[1][0:64, :], 0.0)

                    estate = {}

                    def emit_scores(b, h):
                        base = b * N
                        par = h % 2
                        p0 = par * 64
                        qsl = qk_sb[h // 2][p0:p0 + 64, base:base + N]
                        nc.vector.tensor_copy(qb[par][p0:p0 + 64, :], qsl)
                        exps = []
                        for i, (kt0, kr) in enumerate(KTILES):
                            ps_s = pss.tile([128, N], F32,
                                            name=f"ss{b}_{h}_{i}", tag="ps_s")
                            for (qc0, qcw) in QCH:
                                nc.tensor.matmul(
                                    ps_s[:kr, qc0:qc0 + qcw],
                                    qk_sb[6 + h // 2][:, base + kt0:
                                                      base + kt0 + kr],
                                    qb[par][:, qc0:qc0 + qcw],
                                    start=True, stop=True)
                            e = pexp.tile([128, N], BF16,
                                          name=f"e{b}_{h}_{i}", tag=f"e{i}")
                            nc.scalar.activation(e[:kr, :], ps_s[:kr, :],
                                                 AF.Exp)
                            exps.append((e, kr))
                        estate[(b, h)] = exps

                    def emit_av(b, h, den):
                        exps = estate.pop((b, h))
                        ps_o = pso.tile([65, N], F32, name=f"po{b}_{h}",
                                        tag="ps_o")
                        for (qc0, qcw) in QCH:
                            for i, (e, kr) in enumerate(exps):
                                nc.tensor.matmul(
                                    ps_o[:, qc0:qc0 + qcw],
                                    vbuf[(b, i)][:kr, h * 65:(h + 1) * 65],
                                    e[:kr, qc0:qc0 + qcw],
                                    start=(i == 0), stop=(i == 4))
                        dr = (h % 2) * 32
                        nc.vector.tensor_copy(den[dr:dr + 1, :],
                                              ps_o[64:65, :])
                        oU = o_tiles[b][h // 2]
                        p0 = (h % 2) * 64
                        nc.vector.tensor_copy(oU[p0:p0 + 64, :], ps_o[0:64, :])

                    def emit_pair_tail(b, j, den):
                        # den [33, N]: pair's softmax denominators on
                        # partitions 0 and 32 (quadrant-aligned for the DVE);
                        # normalize o_tiles[b][j] in place. partition_broadcast
                        # only supports (src partition 0, dst base 0), so the
                        # odd head's reciprocal is first copied down to
                        # partition 0 and each 64-row half is normalized by
                        # its own base-0 bc tile.
                        rec = p2t.tile([33, N], BF16, name=f"rc{b}_{j}",
                                       tag="rec")
                        with nc.allow_low_precision(reason="softmax denom"):
                            nc.vector.reciprocal(rec[:], den[:])
                        rec2 = p2t.tile([1, N], BF16, name=f"r2{b}_{j}",
                                        tag="rec2")
                        nc.vector.tensor_copy(rec2[:], rec[32:33, :])
                        bc = p2t.tile([128, N], BF16, name=f"bc{b}_{j}",
                                      tag=f"bc{j % 2}")
                        nc.gpsimd.partition_broadcast(bc[0:64, :],
                                                      rec[0:1, :])
                        bcu = p2t.tile([64, N], BF16, name=f"bu{b}_{j}",
                                       tag=f"bu{j % 2}")
                        nc.gpsimd.partition_broadcast(bcu[:], rec2[0:1, :])
                        nc.vector.tensor_copy(bc[64:128, :], bcu[:])
                        nc.vector.tensor_tensor(o_tiles[b][j][:],
                                                o_tiles[b][j][:], bc[:],
                                                ALU.mult)

                    def emit_proj(b):
                        base = b * N
                        for n in range(6):
                            pp = pss.tile([128, N], F32, name=f"pp{b}_{n}",
                                          tag="ps_s")
                            for (qc0, qcw) in QCH:
                                for j in range(6):
                                    nc.tensor.matmul(
                                        pp[:, qc0:qc0 + qcw],
                                        wp_sb[j][:, n * 128:(n + 1) * 128],
                                        o_tiles[b][j][:, qc0:qc0 + qcw],
                                        start=(j == 0), stop=(j == 5))
                            xr = p2t.tile([128, N], BF16, name=f"xr{b}_{n}",
                                          tag=f"xr{n % 2}")
                            nc.sync.dma_start(
                                out=xr[:],
                                in_=xT_d[n * 128:(n + 1) * 128,
                                         base:base + N])
                            nc.vector.scalar_tensor_tensor(
                                x2[n][:, base:base + N], pp[:],
                                pb[:, n:n + 1], xr[:], ALU.add, ALU.add)

                    o_tiles = {}
                    dens = {}
                    seq = [(b, h) for b in range(BPC) for h in range(H)]

                    def emit_av_tail(b, h):
                        if h % 2 == 0:
                            dens[b] = p2t.tile([33, N], F32, name=f"dn{b}_{h}",
                                               tag="den")
                            nc.vector.memset(dens[b][0:32, :], 1.0)
                        emit_av(b, h, dens[b])
                        if h % 2 == 1:
                            emit_pair_tail(b, h // 2, dens.pop(b))

                    for idx, (b, h) in enumerate(seq):
                        if h == 0:
                            o_tiles[b] = [oal.tile([128, N], BF16,
                                                   name=f"oa{b}_{j}",
                                                   tag=f"oa{j}")
                                          for j in range(6)]
                        emit_scores(b, h)
                        if idx > 0:
                            emit_av_tail(*seq[idx - 1])
                        if idx > 1:
                            qb_, qh_ = seq[idx - 2]
                            if qh_ == H - 1:
                                emit_proj(qb_)
                    emit_av_tail(*seq[-1])
                    emit_proj(seq[-1][0])

            # ---------------- P4: LN2 + MLP ----------------
            with tc.tile_pool(name="w12", bufs=1) as w12, \
                 tc.tile_pool(name="p4x", bufs=2) as p4x, \
                 tc.tile_pool(name="p4s", bufs=2) as p4s, \
                 tc.tile_pool(name="h2p", bufs=2) as h2p, \
                 tc.tile_pool(name="p4t", bufs=2) as p4t, \
                 tc.tile_pool(name="pgl", bufs=1) as pgl, \
                 tc.tile_pool(name="ps4s", bufs=1, space="PSUM") as ps4s, \
                 tc.tile_pool(name="ps41", bufs=3, space="PSUM") as ps41, \
                 tc.tile_pool(name="ps42", bufs=2, space="PSUM") as ps42:
                w1_sb = [w12.tile([128, HID], BF16, name=f"w1_{k}")
                         for k in range(6)]
                for k in range(6):
                    nc.sync.dma_start(out=w1_sb[k][:],
                                      in_=w1_d[k * 128:(k + 1) * 128, :])
                w2_sb = [w12.tile([128, C], BF16, name=f"w2_{k}")
                         for k in range(24)]
                h2c = {}

                def emit_stats2(ci):
                    c0, cw = CHUNKS[ci]
                    ps_sum = ps4s.tile([128, cw], F32, name=f"2ss_{c0}",
                                       tag="ps_sum2")
                    ps_ssq = ps4s.tile([128, cw], F32, name=f"2sq_{c0}",
                                       tag="ps_ssq2")
                    for k in range(6):
                        xq = p4x.tile([128, cw], BF16, name=f"2xq{k}_{c0}",
                                      tag=f"2xq{k}")
                        nc.vector.tensor_tensor(xq[:], x2[k][:, c0:c0 + cw],
                                                x2[k][:, c0:c0 + cw], ALU.mult)
                        nc.tensor.matmul(ps_sum[:], ones128[:],
                                         x2[k][:, c0:c0 + cw],
                                         start=(k == 0), stop=(k == 5))
                        nc.tensor.matmul(ps_ssq[:], ones128[:], xq[:],
                                         start=(k == 0), stop=(k == 5))
                    mu = p4s.tile([128, cw], F32, name=f"2mu_{c0}", tag="2mu")
                    nc.vector.tensor_scalar(mu[:], ps_sum[:], 1.0 / C, None,
                                            ALU.mult)
                    t1 = p4s.tile([128, cw], F32, name=f"2t1_{c0}", tag="2tt")
                    nc.vector.tensor_tensor(t1[:], ps_sum[:], mu[:], ALU.mult)
                    t2 = p4s.tile([128, cw], F32, name=f"2t2_{c0}", tag="2tt")
                    nc.vector.tensor_tensor(t2[:], ps_ssq[:], t1[:],
                                            ALU.subtract)
                    t3 = p4s.tile([128, cw], F32, name=f"2t3_{c0}", tag="2tt")
                    nc.vector.tensor_scalar(t3[:], t2[:], 1.0 / C, 1e-5,
                                            ALU.mult, ALU.add)
                    lnv = p4s.tile([128, cw], F32, name=f"2lv_{c0}", tag="2tt")
                    nc.scalar.activation(lnv[:], t3[:], AF.Ln)
                    rs = p4s.tile([128, cw], BF16, name=f"2rs_{c0}", tag="2rs")
                    nc.scalar.activation(rs[:], lnv[:], AF.Exp, scale=-0.5)
                    murs = p4s.tile([128, cw], BF16, name=f"2mr_{c0}",
                                    tag="2mr")
                    nc.vector.tensor_tensor(murs[:], mu[:], rs[:], ALU.mult)
                    hs = []
                    for k in range(6):
                        tmp = p4s.tile([128, cw], F32, name=f"2tp{k}_{c0}",
                                       tag="2tp")
                        nc.vector.tensor_tensor(tmp[:], x2[k][:, c0:c0 + cw],
                                                rs[:], ALU.mult)
                        ht = h2p.tile([128, cw], BF16, name=f"2h{k}_{c0}",
                                      tag=f"2h{k}")
                        nc.vector.tensor_tensor(ht[:], tmp[:], murs[:],
                                                ALU.subtract)
                        hs.append(ht)
                    h2c[ci] = hs

                emit_stats2(0)
                for ci in range(5):
                    c0, cw = CHUNKS[ci]
                    hs = h2c.pop(ci)
                    gl = []
                    for n1 in range(24):
                        p1p = ps41.tile([128, cw], F32, name=f"p41_{n1}_{c0}",
                                        tag="ps41")
                        for k in range(6):
                            nc.tensor.matmul(
                                p1p[:], w1_sb[k][:, n1 * 128:(n1 + 1) * 128],
                                hs[k][:], start=(k == 0), stop=(k == 5))
                        g = pgl.tile([128, cw], BF16, name=f"gl{n1}_{c0}",
                                     tag=f"gl{n1}")
                        nc.scalar.activation(g[:], p1p[:], AF.Gelu,
                                             bias=b1a[:, n1:n1 + 1])
                        gl.append(g)
                    if ci == 0:
                        for k in range(24):
                            nc.sync.dma_start(
                                out=w2_sb[k][:],
                                in_=w2_d[k * 128:(k + 1) * 128, :])
                    if ci + 1 < 5:
                        emit_stats2(ci + 1)
                    for n2 in range(6):
                        p2p = ps42.tile([128, cw], F32, name=f"p42_{n2}_{c0}",
                                        tag="ps42")
                        for k2 in range(24):
                            nc.tensor.matmul(
                                p2p[:], w2_sb[k2][:, n2 * 128:(n2 + 1) * 128],
                                gl[k2][:], start=(k2 == 0), stop=(k2 == 23))
                        oo = p4t.tile([128, cw], F32, name=f"oo{n2}_{c0}",
                                      tag="oo")
                        nc.vector.scalar_tensor_tensor(
                            oo[:], p2p[:], b2a[:, n2:n2 + 1],
                            x2[n2][:, c0:c0 + cw], ALU.add, ALU.add)
                        nc.sync.dma_start(
                            out=outT_d[n2 * 128:(n2 + 1) * 128, c0:c0 + cw],
                            in_=oo[:])
    nc.compile()
    return nc


_CACHE = {}


def _prep_shared(inputs):
    f32 = np.float32
    bf = ml_dtypes.bfloat16
    qkv_w = np.asarray(inputs["qkv_w"], f32)
    ln1_g = np.asarray(inputs["ln1_g"], f32)
    ln1_b = np.asarray(inputs["ln1_b"], f32)
    qkv_b = np.asarray(inputs["qkv_b"], f32)
    W = qkv_w * ln1_g[:, None]
    bq = ln1_b @ qkv_w + qkv_b
    W = W.copy()
    W[:, :C] *= 0.125
    bq = bq.copy()
    bq[:C] *= 0.125

    proj_w = np.asarray(inputs["proj_w"], f32)
    fc1_w = np.asarray(inputs["fc1_w"], f32)
    ln2_g = np.asarray(inputs["ln2_g"], f32)
    ln2_b = np.asarray(inputs["ln2_b"], f32)
    fc1_b = np.asarray(inputs["fc1_b"], f32)
    W1 = fc1_w * ln2_g[:, None]
    b1 = ln2_b @ fc1_w + fc1_b
    fc2_w = np.asarray(inputs["fc2_w"], f32)

    return {
        "wqkv": np.ascontiguousarray(W.astype(bf)),
        "qkb": np.ascontiguousarray(bq[:2 * C].reshape(12, 128).T.astype(f32)),
        "vbb": np.ascontiguousarray(np.tile(bq[2 * C:], (128, 1)).astype(f32)),
        "wp": np.ascontiguousarray(proj_w.reshape(6, 128, C).astype(bf)),
        "pb": np.ascontiguousarray(
            np.asarray(inputs["proj_b"], f32).reshape(6, 128).T),
        "w1": np.ascontiguousarray(W1.astype(bf)),
        "b1a": np.ascontiguousarray(b1.reshape(24, 128).T.astype(f32)),
        "w2": np.ascontiguousarray(fc2_w.astype(bf)),
        "b2a": np.ascontiguousarray(
            np.asarray(inputs["fc2_b"], f32).reshape(6, 128).T),
    }


def _make_in_maps(inputs):
    bf = ml_dtypes.bfloat16
    x = np.asarray(inputs["x"], np.float32)
    shared = _prep_shared(inputs)
    in_maps = []
    for c in range(NCORES):
        xT = np.ascontiguousarray(
            x[c * BPC:(c + 1) * BPC].reshape(T, C).T.astype(bf))
        m = {"xT": xT}
        m.update(shared)
        in_maps.append(m)
    return in_maps


def kernel(**inputs):
    if "nc" not in _CACHE:
        _CACHE["nc"] = _build_nc()
    nc = _CACHE["nc"]
    in_maps = _make_in_maps(inputs)
    res = run_bass_kernel_spmd(nc, in_maps, list(range(NCORES)))
    out = np.empty((B, N, C), np.float32)
    for c in range(NCORES):
        outT = res.results[c]["outT"]
        out[c * BPC:(c + 1) * BPC] = outT.T.reshape(BPC, N, C)
    return out


# revision 22
# speedup vs baseline: 1.3378x; 1.0177x over previous
"""Fused transformer block (pre-norm attn + MLP) for Trainium2, 8 cores.

Sharding: data-parallel over batch (32 batches -> 4 per core), no
collectives. Each core computes the full block on its shard.

v2 design notes (vs v1 baseline at 1051us):
- Every matmul runs in the PE's 128x128 tile mode (K=128 contraction or
  round-up): scores use a zero-padded q buffer so the K=64 head_dim
  contraction becomes K=128 with junk-times-zero rows; LN stats use an
  all-ones [128,128] stationary so the column sums come out broadcast
  across all 128 psum partitions (no 1-row matmuls, no gpsimd
  partition-broadcast for LN). Mixed tile modes force PE drains and kept
  the HAM clock gate at K=4/8 (1.2GHz) for the entire attention phase in
  v1 (440us window at half clock).
- x streams in bf16 (host-cast); LN apply reads bf16 x directly.
- proj contracts head PAIRS (o stacked [128,N]) -> K=128, half the
  matmuls of per-head K=64.
- softmax denominators ride attn@v as psum row 64 (ones column in v);
  all 12 heads' denominators are copied into one [12,N] tile and
  reciprocal'd in ONE DVE instruction (v1 spent 45us/batch on [1,N]
  reciprocals).
- x2 (attn residual) stays SBUF-resident in bf16; LN2 stats/apply are
  pipelined per chunk inside the MLP phase (v1 had a 90us P3 phase with
  an idle PE and a DRAM round trip).
- Phases software-pipeline: LN stats of chunk c+1 are emitted before the
  qkv matmuls of chunk c; scores of head h+1 before attn@v of head h;
  the first scores of batch b+1 before proj of batch b; MLP stats of
  chunk c+1 between FC1(c) and FC2(c).
LN gains/biases and the attention scale are folded into the weights on
the host. All matmul operands bf16; psum f32. ACT exp has no max
subtraction (|scores| < 3). Ln/Exp share one ACT table set; Gelu only
appears in the MLP phase (one table switch).
"""
import numpy as np
import ml_dtypes
import concourse.bacc as bacc
import concourse.mybir as mybir
import concourse.tile as tile
from concourse.bass_utils import run_bass_kernel_spmd

F32 = mybir.dt.float32
BF16 = mybir.dt.bfloat16
AF = mybir.ActivationFunctionType
ALU = mybir.AluOpType

B, N, C = 32, 577, 768
H, D = 12, 64
HID = 3072
NCORES = 8
BPC = B // NCORES            # 4 batches per core
T = BPC * N                  # 2308 tokens per core
CHUNKS = [(0, 512), (512, 512), (1024, 512), (1536, 512), (2048, 260)]
# P1 chunks are batch-aligned so every v-evac psum->vbuf copy starts at
# partition 0 (DVE ops with >64 channels must be quadrant-aligned).
P1CH = [(b * 577 + o, w) for b in range(4) for (o, w) in ((0, 512), (512, 65))]
KTILES = [(0, 128), (128, 128), (256, 128), (384, 128), (512, 65)]
QCH = [(0, 512), (512, 65)]  # free-dim split of 577 (psum bank = 512 f32)


def _build_nc():
    nc = bacc.Bacc("TRN2", target_bir_lowering=False, debug=False,
                   num_devices=NCORES)
    xT_d = nc.dram_tensor("xT", [C, T], BF16, kind="ExternalInput")
    wqkv_d = nc.dram_tensor("wqkv", [C, 3 * C], BF16, kind="ExternalInput")
    qkb_d = nc.dram_tensor("qkb", [128, 12], F32, kind="ExternalInput")
    vbb_d = nc.dram_tensor("vbb", [128, C], F32, kind="ExternalInput")
    wp_d = nc.dram_tensor("wp", [6, 128, C], BF16, kind="ExternalInput")
    pb_d = nc.dram_tensor("pb", [128, 6], F32, kind="ExternalInput")
    w1_d = nc.dram_tensor("w1", [C, HID], BF16, kind="ExternalInput")
    b1a_d = nc.dram_tensor("b1a", [128, 24], F32, kind="ExternalInput")
    w2_d = nc.dram_tensor("w2", [HID, C], BF16, kind="ExternalInput")
    b2a_d = nc.dram_tensor("b2a", [128, 6], F32, kind="ExternalInput")
    outT_d = nc.dram_tensor("outT", [C, T], F32, kind="ExternalOutput")

    with tile.TileContext(nc) as tc:
        with tc.tile_pool(name="cst", bufs=1) as cst, \
             tc.tile_pool(name="x2p", bufs=1) as x2p:
            ones128 = cst.tile([128, 128], BF16)
            nc.vector.memset(ones128[:], 1.0)
            qkb = cst.tile([128, 12], F32)
            nc.sync.dma_start(out=qkb[:], in_=qkb_d[:])
            vbb = cst.tile([128, C], F32)
            nc.sync.dma_start(out=vbb[:], in_=vbb_d[:])
            pb = cst.tile([128, 6], F32)
            nc.sync.dma_start(out=pb[:], in_=pb_d[:])
            b1a = cst.tile([128, 24], F32)
            nc.sync.dma_start(out=b1a[:], in_=b1a_d[:])
            b2a = cst.tile([128, 6], F32)
            nc.sync.dma_start(out=b2a[:], in_=b2a_d[:])
            x2 = [x2p.tile([128, T], BF16, name=f"x2_{k}") for k in range(6)]

            with tc.tile_pool(name="qks", bufs=1) as qks, \
                 tc.tile_pool(name="vbp", bufs=1) as vbp:
                qk_sb = [qks.tile([128, T], BF16, name=f"qk{n}")
                         for n in range(12)]
                vbuf = {}
                for b in range(BPC):
                    for i in range(5):
                        vbuf[(b, i)] = vbp.tile([128, H * 65], BF16,
                                                name=f"vb{b}_{i}")
                        ocol = vbuf[(b, i)].rearrange(
                            "p (h e) -> p h e", e=65)[:, :, 64]
                        nc.vector.memset(ocol, 1.0)

                # ---------------- P1: LN1 + qk + v ----------------
                with tc.tile_pool(name="p1w", bufs=1) as p1w:
                    wq = [p1w.tile([128, 3 * C], BF16, name=f"wq{k}")
                          for k in range(6)]
                    with tc.tile_pool(name="p1x", bufs=2) as p1x, \
                         tc.tile_pool(name="p1h", bufs=2) as p1h, \
                         tc.tile_pool(name="p1s", bufs=2) as p1s, \
                         tc.tile_pool(name="ps1", bufs=2, space="PSUM") as ps1, \
                         tc.tile_pool(name="psqk", bufs=2, space="PSUM") as psqk, \
                         tc.tile_pool(name="psv", bufs=1, space="PSUM") as psv:
                        xc = {}
                        h1c = {}

                        def emit_stats(ci):
                            c0, cw = P1CH[ci]
                            xs = []
                            for k in range(6):
                                xt = p1x.tile([128, cw], BF16,
                                              name=f"x{k}_{c0}", tag=f"x{k}")
                                nc.sync.dma_start(
                                    out=xt[:],
                                    in_=xT_d[k * 128:(k + 1) * 128,
                                             c0:c0 + cw])
                                xs.append(xt)
                            xc[ci] = xs
                            ps_sum = ps1.tile([128, cw], F32,
                                              name=f"pss_{c0}", tag="ps_sum")
                            ps_ssq = ps1.tile([128, cw], F32,
                                              name=f"psq_{c0}", tag="ps_ssq")
                            for k in range(6):
                                xq = p1x.tile([128, cw], BF16,
                                              name=f"xq{k}_{c0}", tag=f"xq{k}")
                                nc.vector.tensor_tensor(xq[:], xs[k][:],
                                                        xs[k][:], ALU.mult)
                                nc.tensor.matmul(ps_sum[:], ones128[:],
                                                 xs[k][:],
                                                 start=(k == 0), stop=(k == 5))
                                nc.tensor.matmul(ps_ssq[:], ones128[:], xq[:],
                                                 start=(k == 0), stop=(k == 5))
                            return ps_sum, ps_ssq

                        def emit_ln(ci, ps_sum, ps_ssq):
                            c0, cw = P1CH[ci]
                            mu = p1s.tile([128, cw], F32, name=f"mu_{c0}",
                                          tag="mu")
                            nc.vector.tensor_scalar(mu[:], ps_sum[:], 1.0 / C,
                                                    None, ALU.mult)
                            t1 = p1s.tile([128, cw], F32, name=f"t1_{c0}",
                                          tag="tt")
                            nc.vector.tensor_tensor(t1[:], ps_sum[:], mu[:],
                                                    ALU.mult)
                            t2 = p1s.tile([128, cw], F32, name=f"t2_{c0}",
                                          tag="tt")
                            nc.vector.tensor_tensor(t2[:], ps_ssq[:], t1[:],
                                                    ALU.subtract)
                            t3 = p1s.tile([128, cw], F32, name=f"t3_{c0}",
                                          tag="tt")
                            nc.vector.tensor_scalar(t3[:], t2[:], 1.0 / C,
                                                    1e-5, ALU.mult, ALU.add)
                            lnv = p1s.tile([128, cw], F32, name=f"lnv_{c0}",
                                           tag="tt")
                            nc.scalar.activation(lnv[:], t3[:], AF.Ln)
                            rs = p1s.tile([128, cw], BF16, name=f"rs_{c0}",
                                          tag="rs")
                            nc.scalar.activation(rs[:], lnv[:], AF.Exp,
                                                 scale=-0.5)
                            murs = p1s.tile([128, cw], BF16, name=f"mr_{c0}",
                                            tag="mr")
                            nc.vector.tensor_tensor(murs[:], mu[:], rs[:],
                                                    ALU.mult)
                            hs = []
                            for k in range(6):
                                tmp = p1s.tile([128, cw], F32,
                                               name=f"tp{k}_{c0}", tag="tp")
                                nc.vector.tensor_tensor(tmp[:], xc[ci][k][:],
                                                        rs[:], ALU.mult)
                                ht = p1h.tile([128, cw], BF16,
                                              name=f"h{k}_{c0}", tag=f"h{k}")
                                nc.vector.tensor_tensor(ht[:], tmp[:], murs[:],
                                                        ALU.subtract)
                                hs.append(ht)
                            h1c[ci] = hs

                        def emit_qkv(ci):
                            c0, cw = P1CH[ci]
                            hs = h1c[ci]
                            for n in range(12):
                                pq = psqk.tile([128, cw], F32,
                                               name=f"pq{n}_{c0}", tag="psqk")
                                for k in range(6):
                                    nc.tensor.matmul(
                                        pq[:], wq[k][:, n * 128:(n + 1) * 128],
                                        hs[k][:], start=(k == 0), stop=(k == 5))
                                if n < 6:
                                    nc.scalar.activation(
                                        qk_sb[n][:, c0:c0 + cw], pq[:],
                                        AF.Identity, bias=qkb[:, n:n + 1])
                                else:
                                    nc.vector.tensor_scalar(
                                        qk_sb[n][:, c0:c0 + cw], pq[:],
                                        qkb[:, n:n + 1], None, ALU.add)
                            # v for this chunk (chunks are batch-aligned,
                            # so each KTILE lies fully inside one chunk and
                            # all partition bases are 0)
                            b = c0 // N
                            o0 = c0 - b * N
                            for i, (kt0, kr) in enumerate(KTILES):
                                if kt0 < o0 or kt0 >= o0 + cw:
                                    continue
                                lt = kt0 - o0
                                pv = psv.tile([128, C], F32,
                                              name=f"pv_{c0}_{lt}", tag="psv")
                                for k in range(6):
                                    nc.tensor.matmul(
                                        pv[:kr, 0:512],
                                        hs[k][:, lt:lt + kr],
                                        wq[k][:, 1536:2048],
                                        start=(k == 0), stop=(k == 5))
                                for k in range(6):
                                    nc.tensor.matmul(
                                        pv[:kr, 512:768],
                                        hs[k][:, lt:lt + kr],
                                        wq[k][:, 2048:2304],
                                        start=(k == 0), stop=(k == 5))
                                vm = vbuf[(b, i)]
                                dst = vm[0:kr].rearrange(
                                    "p (h e) -> p h e", e=65)[:, :, 0:64]
                                srcv = pv[0:kr].rearrange(
                                    "p (h e) -> p h e", e=64)
                                bia = vbb[0:kr].rearrange(
                                    "p (h e) -> p h e", e=64)
                                nc.vector.tensor_tensor(dst, srcv, bia,
                                                        ALU.add)
                            del h1c[ci], xc[ci]

                        emit_ln(0, *emit_stats(0))
                        for k in range(6):
                            nc.sync.dma_start(
                                out=wq[k][:],
                                in_=wqkv_d[k * 128:(k + 1) * 128, :])
                        emit_ln(1, *emit_stats(1))
                        for ci in range(len(P1CH)):
                            emit_qkv(ci)
                            if ci + 2 < len(P1CH):
                                emit_ln(ci + 2, *emit_stats(ci + 2))

                # ---------------- P2: attention + proj ----------------
                with tc.tile_pool(name="wpp", bufs=1) as wpp, \
                     tc.tile_pool(name="qbp", bufs=1) as qbp, \
                     tc.tile_pool(name="oal", bufs=2) as oal, \
                     tc.tile_pool(name="p2t", bufs=2) as p2t, \
                     tc.tile_pool(name="pexp", bufs=2) as pexp, \
                     tc.tile_pool(name="pss", bufs=2, space="PSUM") as pss, \
                     tc.tile_pool(name="pso", bufs=2, space="PSUM") as pso:
                    wp_sb = [wpp.tile([128, C], BF16, name=f"wp{j}")
                             for j in range(6)]
                    for j in range(6):
                        nc.sync.dma_start(out=wp_sb[j][:], in_=wp_d[j])
                    qb = [qbp.tile([128, N], BF16, name=f"qb{par}")
                          for par in range(2)]
                    nc.vector.memset(qb[0][64:128, :], 0.0)
                    nc.vector.memset(qb[1][0:64, :], 0.0)

                    estate = {}

                    def emit_scores(b, h):
                        base = b * N
                        par = h % 2
                        p0 = par * 64
                        qsl = qk_sb[h // 2][p0:p0 + 64, base:base + N]
                        nc.vector.tensor_copy(qb[par][p0:p0 + 64, :], qsl)
                        exps = []
                        for i, (kt0, kr) in enumerate(KTILES):
                            ps_s = pss.tile([128, N], F32,
                                            name=f"ss{b}_{h}_{i}", tag="ps_s")
                            for (qc0, qcw) in QCH:
                                nc.tensor.matmul(
                                    ps_s[:kr, qc0:qc0 + qcw],
                                    qk_sb[6 + h // 2][:, base + kt0:
                                                      base + kt0 + kr],
                                    qb[par][:, qc0:qc0 + qcw],
                                    start=True, stop=True)
                            e = pexp.tile([128, N], BF16,
                                          name=f"e{b}_{h}_{i}", tag=f"e{i}")
                            nc.scalar.activation(e[:kr, :], ps_s[:kr, :],
                                                 AF.Exp)
                            exps.append((e, kr))
                        estate[(b, h)] = exps

                    def emit_av(b, h, den):
                        exps = estate.pop((b, h))
                        ps_o = pso.tile([65, N], F32, name=f"po{b}_{h}",
                                        tag="ps_o")
                        for (qc0, qcw) in QCH:
                            for i, (e, kr) in enumerate(exps):
                                nc.tensor.matmul(
                                    ps_o[:, qc0:qc0 + qcw],
                                    vbuf[(b, i)][:kr, h * 65:(h + 1) * 65],
                                    e[:kr, qc0:qc0 + qcw],
                                    start=(i == 0), stop=(i == 4))
                        dr = (h % 4) * 32
                        nc.vector.tensor_copy(den[dr:dr + 1, :],
                                              ps_o[64:65, :])
                        oU = o_tiles[b][h // 2]
                        p0 = (h % 2) * 64
                        nc.vector.tensor_copy(oU[p0:p0 + 64, :], ps_o[0:64, :])

                    def emit_group_tail(b, g, den):
                        # den [97, N]: 4 heads' softmax denominators on
                        # quadrant partitions 0/32/64/96 (DVE reciprocal cost
                        # is flat in the partition count, so batch 4 heads per
                        # instruction). partition_broadcast only supports
                        # (src partition 0, dst base 0), so rows 32/64/96 are
                        # first copied down to partition 0.
                        rec = p2t.tile([97, N], BF16, name=f"rc{b}_{g}",
                                       tag="rec")
                        with nc.allow_low_precision(reason="softmax denom"):
                            nc.vector.reciprocal(rec[:], den[:])
                        srcs = {0: rec}
                        for r in (32, 64, 96):
                            t = p2t.tile([1, N], BF16, name=f"rx{b}_{g}_{r}",
                                         tag=f"rx{r}")
                            nc.vector.tensor_copy(t[:], rec[r:r + 1, :])
                            srcs[r] = t
                        for jj in range(2):
                            j = 2 * g + jj
                            bc = p2t.tile([128, N], BF16, name=f"bc{b}_{j}",
                                          tag=f"bc{j % 2}")
                            nc.gpsimd.partition_broadcast(
                                bc[0:64, :], srcs[jj * 64][0:1, :])
                            bcu = p2t.tile([64, N], BF16, name=f"bu{b}_{j}",
                                           tag=f"bu{j % 2}")
                            nc.gpsimd.partition_broadcast(
                                bcu[:], srcs[jj * 64 + 32][0:1, :])
                            nc.vector.tensor_copy(bc[64:128, :], bcu[:])
                            nc.vector.tensor_tensor(o_tiles[b][j][:],
                                                    o_tiles[b][j][:], bc[:],
                                                    ALU.mult)

                    def emit_proj(b):
                        base = b * N
                        for n in range(6):
                            pp = pss.tile([128, N], F32, name=f"pp{b}_{n}",
                                          tag="ps_s")
                            for (qc0, qcw) in QCH:
                                for j in range(6):
                                    nc.tensor.matmul(
                                        pp[:, qc0:qc0 + qcw],
                                        wp_sb[j][:, n * 128:(n + 1) * 128],
                                        o_tiles[b][j][:, qc0:qc0 + qcw],
                                        start=(j == 0), stop=(j == 5))
                            xr = p2t.tile([128, N], BF16, name=f"xr{b}_{n}",
                                          tag=f"xr{n % 2}")
                            nc.sync.dma_start(
                                out=xr[:],
                                in_=xT_d[n * 128:(n + 1) * 128,
                                         base:base + N])
                            nc.vector.scalar_tensor_tensor(
                                x2[n][:, base:base + N], pp[:],
                                pb[:, n:n + 1], xr[:], ALU.add, ALU.add)

                    o_tiles = {}
                    dens = {}
                    seq = [(b, h) for b in range(BPC) for h in range(H)]

                    def emit_av_tail(b, h):
                        if h % 4 == 0:
                            dens[b] = p2t.tile([97, N], F32, name=f"dn{b}_{h}",
                                               tag="den")
                            nc.vector.memset(dens[b][0:96, :], 1.0)
                        emit_av(b, h, dens[b])
                        if h % 4 == 3:
                            emit_group_tail(b, h // 4, dens.pop(b))

                    for idx, (b, h) in enumerate(seq):
                        if h == 0:
                            o_tiles[b] = [oal.tile([128, N], BF16,
                                                   name=f"oa{b}_{j}",
                                                   tag=f"oa{j}")
                                          for j in range(6)]
                        emit_scores(b, h)
                        if idx > 0:
                            emit_av_tail(*seq[idx - 1])
                        if idx > 2:
                            qb_, qh_ = seq[idx - 3]
                            if qh_ == H - 1:
                                emit_proj(qb_)
                    emit_av_tail(*seq[-1])
                    emit_proj(seq[-1][0])

            # ---------------- P4: LN2 + MLP ----------------
            with tc.tile_pool(name="w12", bufs=1) as w12, \
                 tc.tile_pool(name="p4x", bufs=2) as p4x, \
                 tc.tile_pool(name="p4s", bufs=2) as p4s, \
                 tc.tile_pool(name="h2p", bufs=2) as h2p, \
                 tc.tile_pool(name="p4t", bufs=2) as p4t, \
                 tc.tile_pool(name="pgl", bufs=1) as pgl, \
                 tc.tile_pool(name="ps4s", bufs=1, space="PSUM") as ps4s, \
                 tc.tile_pool(name="ps41", bufs=3, space="PSUM") as ps41, \
                 tc.tile_pool(name="ps42", bufs=2, space="PSUM") as ps42:
                w1_sb = [w12.tile([128, HID], BF16, name=f"w1_{k}")
                         for k in range(6)]
                for k in range(6):
                    nc.sync.dma_start(out=w1_sb[k][:],
                                      in_=w1_d[k * 128:(k + 1) * 128, :])
                w2_sb = [w12.tile([128, C], BF16, name=f"w2_{k}")
                         for k in range(24)]
                h2c = {}

                def emit_stats2(ci):
                    c0, cw = CHUNKS[ci]
                    ps_sum = ps4s.tile([128, cw], F32, name=f"2ss_{c0}",
                                       tag="ps_sum2")
                    ps_ssq = ps4s.tile([128, cw], F32, name=f"2sq_{c0}",
                                       tag="ps_ssq2")
                    for k in range(6):
                        xq = p4x.tile([128, cw], BF16, name=f"2xq{k}_{c0}",
                                      tag=f"2xq{k}")
                        nc.vector.tensor_tensor(xq[:], x2[k][:, c0:c0 + cw],
                                                x2[k][:, c0:c0 + cw], ALU.mult)
                        nc.tensor.matmul(ps_sum[:], ones128[:],
                                         x2[k][:, c0:c0 + cw],
                                         start=(k == 0), stop=(k == 5))
                        nc.tensor.matmul(ps_ssq[:], ones128[:], xq[:],
                                         start=(k == 0), stop=(k == 5))
                    mu = p4s.tile([128, cw], F32, name=f"2mu_{c0}", tag="2mu")
                    nc.vector.tensor_scalar(mu[:], ps_sum[:], 1.0 / C, None,
                                            ALU.mult)
                    t1 = p4s.tile([128, cw], F32, name=f"2t1_{c0}", tag="2tt")
                    nc.vector.tensor_tensor(t1[:], ps_sum[:], mu[:], ALU.mult)
                    t2 = p4s.tile([128, cw], F32, name=f"2t2_{c0}", tag="2tt")
                    nc.vector.tensor_tensor(t2[:], ps_ssq[:], t1[:],
                                            ALU.subtract)
                    t3 = p4s.tile([128, cw], F32, name=f"2t3_{c0}", tag="2tt")
                    nc.vector.tensor_scalar(t3[:], t2[:], 1.0 / C, 1e-5,
                                            ALU.mult, ALU.add)
                    lnv = p4s.tile([128, cw], F32, name=f"2lv_{c0}", tag="2tt")
                    nc.scalar.activation(lnv[:], t3[:], AF.Ln)
                    rs = p4s.tile([128, cw], BF16, name=f"2rs_{c0}", tag="2rs")
                    nc.scalar.activation(rs[:], lnv[:], AF.Exp, scale=-0.5)
                    murs = p4s.tile([128, cw], BF16, name=f"2mr_{c0}",
                                    tag="2mr")
                    nc.vector.tensor_tensor(murs[:], mu[:], rs[:], ALU.mult)
                    hs = []
                    for k in range(6):
                        tmp = p4s.tile([128, cw], F32, name=f"2tp{k}_{c0}",
                                       tag="2tp")
                        nc.vector.tensor_tensor(tmp[:], x2[k][:, c0:c0 + cw],
                                                rs[:], ALU.mult)
                        ht = h2p.tile([128, cw], BF16, name=f"2h{k}_{c0}",
                                      tag=f"2h{k}")
                        nc.vector.tensor_tensor(ht[:], tmp[:], murs[:],
                                                ALU.subtract)
                        hs.append(ht)
                    h2c[ci] = hs

                emit_stats2(0)
                for ci in range(5):
                    c0, cw = CHUNKS[ci]
                    hs = h2c.pop(ci)
                    gl = []
                    for n1 in range(24):
                        p1p = ps41.tile([128, cw], F32, name=f"p41_{n1}_{c0}",
                                        tag="ps41")
                        for k in range(6):
                            nc.tensor.matmul(
                                p1p[:], w1_sb[k][:, n1 * 128:(n1 + 1) * 128],
                                hs[k][:], start=(k == 0), stop=(k == 5))
                        g = pgl.tile([128, cw], BF16, name=f"gl{n1}_{c0}",
                                     tag=f"gl{n1}")
                        nc.scalar.activation(g[:], p1p[:], AF.Gelu,
                                             bias=b1a[:, n1:n1 + 1])
                        gl.append(g)
                    if ci == 0:
                        for k in range(24):
                            nc.sync.dma_start(
                                out=w2_sb[k][:],
                                in_=w2_d[k * 128:(k + 1) * 128, :])
                    if ci + 1 < 5:
                        emit_stats2(ci + 1)
                    for n2 in range(6):
                        p2p = ps42.tile([128, cw], F32, name=f"p42_{n2}_{c0}",
                                        tag="ps42")
                        for k2 in range(24):
                            nc.tensor.matmul(
                                p2p[:], w2_sb[k2][:, n2 * 128:(n2 + 1) * 128],
                                gl[k2][:], start=(k2 == 0), stop=(k2 == 23))
                        oo = p4t.tile([128, cw], F32, name=f"oo{n2}_{c0}",
                                      tag="oo")
                        nc.vector.scalar_tensor_tensor(
                            oo[:], p2p[:], b2a[:, n2:n2 + 1],
                            x2[n2][:, c0:c0 + cw], ALU.add, ALU.add)
                        nc.sync.dma_start(
                            out=outT_d[n2 * 128:(n2 + 1) * 128, c0:c0 + cw],
                            in_=oo[:])
    nc.compile()
    return nc


_CACHE = {}


def _prep_shared(inputs):
    f32 = np.float32
    bf = ml_dtypes.bfloat16
    qkv_w = np.asarray(inputs["qkv_w"], f32)
    ln1_g = np.asarray(inputs["ln1_g"], f32)
    ln1_b = np.asarray(inputs["ln1_b"], f32)
    qkv_b = np.asarray(inputs["qkv_b"], f32)
    W = qkv_w * ln1_g[:, None]
    bq = ln1_b @ qkv_w + qkv_b
    W = W.copy()
    W[:, :C] *= 0.125
    bq = bq.copy()
    bq[:C] *= 0.125

    proj_w = np.asarray(inputs["proj_w"], f32)
    fc1_w = np.asarray(inputs["fc1_w"], f32)
    ln2_g = np.asarray(inputs["ln2_g"], f32)
    ln2_b = np.asarray(inputs["ln2_b"], f32)
    fc1_b = np.asarray(inputs["fc1_b"], f32)
    W1 = fc1_w * ln2_g[:, None]
    b1 = ln2_b @ fc1_w + fc1_b
    fc2_w = np.asarray(inputs["fc2_w"], f32)

    return {
        "wqkv": np.ascontiguousarray(W.astype(bf)),
        "qkb": np.ascontiguousarray(bq[:2 * C].reshape(12, 128).T.astype(f32)),
        "vbb": np.ascontiguousarray(np.tile(bq[2 * C:], (128, 1)).astype(f32)),
        "wp": np.ascontiguousarray(proj_w.reshape(6, 128, C).astype(bf)),
        "pb": np.ascontiguousarray(
            np.asarray(inputs["proj_b"], f32).reshape(6, 128).T),
        "w1": np.ascontiguousarray(W1.astype(bf)),
        "b1a": np.ascontiguousarray(b1.reshape(24, 128).T.astype(f32)),
        "w2": np.ascontiguousarray(fc2_w.astype(bf)),
        "b2a": np.ascontiguousarray(
            np.asarray(inputs["fc2_b"], f32).reshape(6, 128).T),
    }


def _make_in_maps(inputs):
    bf = ml_dtypes.bfloat16
    x = np.asarray(inputs["x"], np.float32)
    shared = _prep_shared(inputs)
    in_maps = []
    for c in range(NCORES):
        xT = np.ascontiguousarray(
            x[c * BPC:(c + 1) * BPC].reshape(T, C).T.astype(bf))
        m = {"xT": xT}
        m.update(shared)
        in_maps.append(m)
    return in_maps


def kernel(**inputs):
    if "nc" not in _CACHE:
        _CACHE["nc"] = _build_nc()
    nc = _CACHE["nc"]
    in_maps = _make_in_maps(inputs)
    res = run_bass_kernel_spmd(nc, in_maps, list(range(NCORES)))
    out = np.empty((B, N, C), np.float32)
    for c in range(NCORES):
        outT = res.results[c]["outT"]
        out[c * BPC:(c + 1) * BPC] = outT.T.reshape(BPC, N, C)
    return out


# revision 24
# speedup vs baseline: 1.5640x; 1.1690x over previous
"""Fused transformer block (pre-norm attn + MLP) for Trainium2, 8 cores.

Sharding: data-parallel over batch (32 batches -> 4 per core), no
collectives. Each core computes the full block on its shard.

v2 design notes (vs v1 baseline at 1051us):
- Every matmul runs in the PE's 128x128 tile mode (K=128 contraction or
  round-up): scores use a zero-padded q buffer so the K=64 head_dim
  contraction becomes K=128 with junk-times-zero rows; LN stats use an
  all-ones [128,128] stationary so the column sums come out broadcast
  across all 128 psum partitions (no 1-row matmuls, no gpsimd
  partition-broadcast for LN). Mixed tile modes force PE drains and kept
  the HAM clock gate at K=4/8 (1.2GHz) for the entire attention phase in
  v1 (440us window at half clock).
- x streams in bf16 (host-cast); LN apply reads bf16 x directly.
- proj contracts head PAIRS (o stacked [128,N]) -> K=128, half the
  matmuls of per-head K=64.
- softmax denominators ride attn@v as psum row 64 (ones column in v);
  all 12 heads' denominators are copied into one [12,N] tile and
  reciprocal'd in ONE DVE instruction (v1 spent 45us/batch on [1,N]
  reciprocals).
- x2 (attn residual) stays SBUF-resident in bf16; LN2 stats/apply are
  pipelined per chunk inside the MLP phase (v1 had a 90us P3 phase with
  an idle PE and a DRAM round trip).
- Phases software-pipeline: LN stats of chunk c+1 are emitted before the
  qkv matmuls of chunk c; scores of head h+1 before attn@v of head h;
  the first scores of batch b+1 before proj of batch b; MLP stats of
  chunk c+1 between FC1(c) and FC2(c).
LN gains/biases and the attention scale are folded into the weights on
the host. All matmul operands bf16; psum f32. ACT exp has no max
subtraction (|scores| < 3). Ln/Exp share one ACT table set; Gelu only
appears in the MLP phase (one table switch).
"""
import numpy as np
import ml_dtypes
import concourse.bacc as bacc
import concourse.mybir as mybir
import concourse.tile as tile
from concourse.bass_utils import run_bass_kernel_spmd

F32 = mybir.dt.float32
BF16 = mybir.dt.bfloat16
F8 = mybir.dt.float8e4
DR = mybir.MatmulPerfMode.DoubleRow
WS = 16.0  # fp8 weight scale (w*WS stored fp8; ACT evac rescales by 1/WS)
AF = mybir.ActivationFunctionType
ALU = mybir.AluOpType

B, N, C = 32, 577, 768
H, D = 12, 64
HID = 3072
NCORES = 8
BPC = B // NCORES            # 4 batches per core
T = BPC * N                  # 2308 tokens per core
CHUNKS = [(0, 512), (512, 512), (1024, 512), (1536, 512), (2048, 260)]
# P1 chunks are batch-aligned so every v-evac psum->vbuf copy starts at
# partition 0 (DVE ops with >64 channels must be quadrant-aligned).
P1CH = [(b * 577 + o, w) for b in range(4) for (o, w) in ((0, 512), (512, 65))]
KTILES = [(0, 128), (128, 128), (256, 128), (384, 128), (512, 65)]
QCH = [(0, 512), (512, 65)]  # free-dim split of 577 (psum bank = 512 f32)


def _build_nc():
    nc = bacc.Bacc("TRN2", target_bir_lowering=False, debug=False,
                   num_devices=NCORES)
    xT_d = nc.dram_tensor("xT", [C, T], BF16, kind="ExternalInput")
    wqkv_d = nc.dram_tensor("wqkv", [C, 3 * C], BF16, kind="ExternalInput")
    qkb_d = nc.dram_tensor("qkb", [128, 12], F32, kind="ExternalInput")
    vbb_d = nc.dram_tensor("vbb", [128, C], F32, kind="ExternalInput")
    wp_d = nc.dram_tensor("wp", [6, 128, C], BF16, kind="ExternalInput")
    pb_d = nc.dram_tensor("pb", [128, 6], F32, kind="ExternalInput")
    w1_d = nc.dram_tensor("w1", [3, 128, 2 * HID], F8, kind="ExternalInput")
    b1a_d = nc.dram_tensor("b1a", [128, 24], F32, kind="ExternalInput")
    w2_d = nc.dram_tensor("w2", [12, 128, 2 * C], F8, kind="ExternalInput")
    b2a_d = nc.dram_tensor("b2a", [128, 6], F32, kind="ExternalInput")
    outT_d = nc.dram_tensor("outT", [C, T], F32, kind="ExternalOutput")

    with tile.TileContext(nc) as tc:
        with tc.tile_pool(name="cst", bufs=1) as cst, \
             tc.tile_pool(name="x2p", bufs=1) as x2p:
            ones128 = cst.tile([128, 128], BF16)
            nc.vector.memset(ones128[:], 1.0)
            qkb = cst.tile([128, 12], F32)
            nc.sync.dma_start(out=qkb[:], in_=qkb_d[:])
            vbb = cst.tile([128, C], F32)
            nc.sync.dma_start(out=vbb[:], in_=vbb_d[:])
            pb = cst.tile([128, 6], F32)
            nc.sync.dma_start(out=pb[:], in_=pb_d[:])
            b1a = cst.tile([128, 24], F32)
            nc.sync.dma_start(out=b1a[:], in_=b1a_d[:])
            b2a = cst.tile([128, 6], F32)
            nc.sync.dma_start(out=b2a[:], in_=b2a_d[:])
            x2 = [x2p.tile([128, T], BF16, name=f"x2_{k}") for k in range(6)]

            with tc.tile_pool(name="qks", bufs=1) as qks, \
                 tc.tile_pool(name="vbp", bufs=1) as vbp:
                qk_sb = [qks.tile([128, T], BF16, name=f"qk{n}")
                         for n in range(12)]
                vbuf = {}
                for b in range(BPC):
                    for i in range(5):
                        vbuf[(b, i)] = vbp.tile([128, H * 65], BF16,
                                                name=f"vb{b}_{i}")
                        ocol = vbuf[(b, i)].rearrange(
                            "p (h e) -> p h e", e=65)[:, :, 64]
                        nc.vector.memset(ocol, 1.0)

                # ---------------- P1: LN1 + qk + v ----------------
                with tc.tile_pool(name="p1w", bufs=1) as p1w:
                    wq = [p1w.tile([128, 3 * C], BF16, name=f"wq{k}")
                          for k in range(6)]
                    with tc.tile_pool(name="p1x", bufs=2) as p1x, \
                         tc.tile_pool(name="p1h", bufs=2) as p1h, \
                         tc.tile_pool(name="p1s", bufs=2) as p1s, \
                         tc.tile_pool(name="ps1", bufs=2, space="PSUM") as ps1, \
                         tc.tile_pool(name="psqk", bufs=2, space="PSUM") as psqk, \
                         tc.tile_pool(name="psv", bufs=1, space="PSUM") as psv:
                        xc = {}
                        h1c = {}

                        def emit_stats(ci):
                            c0, cw = P1CH[ci]
                            xs = []
                            for k in range(6):
                                xt = p1x.tile([128, cw], BF16,
                                              name=f"x{k}_{c0}", tag=f"x{k}")
                                nc.sync.dma_start(
                                    out=xt[:],
                                    in_=xT_d[k * 128:(k + 1) * 128,
                                             c0:c0 + cw])
                                xs.append(xt)
                            xc[ci] = xs
                            ps_sum = ps1.tile([128, cw], F32,
                                              name=f"pss_{c0}", tag="ps_sum")
                            ps_ssq = ps1.tile([128, cw], F32,
                                              name=f"psq_{c0}", tag="ps_ssq")
                            for k in range(6):
                                xq = p1x.tile([128, cw], BF16,
                                              name=f"xq{k}_{c0}", tag=f"xq{k}")
                                nc.vector.tensor_tensor(xq[:], xs[k][:],
                                                        xs[k][:], ALU.mult)
                                nc.tensor.matmul(ps_sum[:], ones128[:],
                                                 xs[k][:],
                                                 start=(k == 0), stop=(k == 5))
                                nc.tensor.matmul(ps_ssq[:], ones128[:], xq[:],
                                                 start=(k == 0), stop=(k == 5))
                            return ps_sum, ps_ssq

                        def emit_ln(ci, ps_sum, ps_ssq):
                            c0, cw = P1CH[ci]
                            mu = p1s.tile([128, cw], F32, name=f"mu_{c0}",
                                          tag="mu")
                            nc.vector.tensor_scalar(mu[:], ps_sum[:], 1.0 / C,
                                                    None, ALU.mult)
                            t1 = p1s.tile([128, cw], F32, name=f"t1_{c0}",
                                          tag="tt")
                            nc.vector.tensor_tensor(t1[:], ps_sum[:], mu[:],
                                                    ALU.mult)
                            t2 = p1s.tile([128, cw], F32, name=f"t2_{c0}",
                                          tag="tt")
                            nc.vector.tensor_tensor(t2[:], ps_ssq[:], t1[:],
                                                    ALU.subtract)
                            t3 = p1s.tile([128, cw], F32, name=f"t3_{c0}",
                                          tag="tt")
                            nc.vector.tensor_scalar(t3[:], t2[:], 1.0 / C,
                                                    1e-5, ALU.mult, ALU.add)
                            lnv = p1s.tile([128, cw], F32, name=f"lnv_{c0}",
                                           tag="tt")
                            nc.scalar.activation(lnv[:], t3[:], AF.Ln)
                            rs = p1s.tile([128, cw], BF16, name=f"rs_{c0}",
                                          tag="rs")
                            nc.scalar.activation(rs[:], lnv[:], AF.Exp,
                                                 scale=-0.5)
                            murs = p1s.tile([128, cw], BF16, name=f"mr_{c0}",
                                            tag="mr")
                            nc.vector.tensor_tensor(murs[:], mu[:], rs[:],
                                                    ALU.mult)
                            hs = []
                            for k in range(6):
                                tmp = p1s.tile([128, cw], F32,
                                               name=f"tp{k}_{c0}", tag="tp")
                                nc.vector.tensor_tensor(tmp[:], xc[ci][k][:],
                                                        rs[:], ALU.mult)
                                ht = p1h.tile([128, cw], BF16,
                                              name=f"h{k}_{c0}", tag=f"h{k}")
                                nc.vector.tensor_tensor(ht[:], tmp[:], murs[:],
                                                        ALU.subtract)
                                hs.append(ht)
                            h1c[ci] = hs

                        def emit_qkv(ci):
                            c0, cw = P1CH[ci]
                            hs = h1c[ci]
                            for n in range(12):
                                pq = psqk.tile([128, cw], F32,
                                               name=f"pq{n}_{c0}", tag="psqk")
                                for k in range(6):
                                    nc.tensor.matmul(
                                        pq[:], wq[k][:, n * 128:(n + 1) * 128],
                                        hs[k][:], start=(k == 0), stop=(k == 5))
                                if n < 6:
                                    nc.scalar.activation(
                                        qk_sb[n][:, c0:c0 + cw], pq[:],
                                        AF.Identity, bias=qkb[:, n:n + 1])
                                else:
                                    nc.vector.tensor_scalar(
                                        qk_sb[n][:, c0:c0 + cw], pq[:],
                                        qkb[:, n:n + 1], None, ALU.add)
                            # v for this chunk (chunks are batch-aligned,
                            # so each KTILE lies fully inside one chunk and
                            # all partition bases are 0)
                            b = c0 // N
                            o0 = c0 - b * N
                            for i, (kt0, kr) in enumerate(KTILES):
                                if kt0 < o0 or kt0 >= o0 + cw:
                                    continue
                                lt = kt0 - o0
                                pv = psv.tile([128, C], F32,
                                              name=f"pv_{c0}_{lt}", tag="psv")
                                for k in range(6):
                                    nc.tensor.matmul(
                                        pv[:kr, 0:512],
                                        hs[k][:, lt:lt + kr],
                                        wq[k][:, 1536:2048],
                                        start=(k == 0), stop=(k == 5))
                                for k in range(6):
                                    nc.tensor.matmul(
                                        pv[:kr, 512:768],
                                        hs[k][:, lt:lt + kr],
                                        wq[k][:, 2048:2304],
                                        start=(k == 0), stop=(k == 5))
                                vm = vbuf[(b, i)]
                                dst = vm[0:kr].rearrange(
                                    "p (h e) -> p h e", e=65)[:, :, 0:64]
                                srcv = pv[0:kr].rearrange(
                                    "p (h e) -> p h e", e=64)
                                bia = vbb[0:kr].rearrange(
                                    "p (h e) -> p h e", e=64)
                                nc.vector.tensor_tensor(dst, srcv, bia,
                                                        ALU.add)
                            del h1c[ci], xc[ci]

                        emit_ln(0, *emit_stats(0))
                        for k in range(6):
                            nc.sync.dma_start(
                                out=wq[k][:],
                                in_=wqkv_d[k * 128:(k + 1) * 128, :])
                        emit_ln(1, *emit_stats(1))
                        for ci in range(len(P1CH)):
                            emit_qkv(ci)
                            if ci + 2 < len(P1CH):
                                emit_ln(ci + 2, *emit_stats(ci + 2))

                # ---------------- P2: attention + proj ----------------
                with tc.tile_pool(name="wpp", bufs=1) as wpp, \
                     tc.tile_pool(name="qbp", bufs=1) as qbp, \
                     tc.tile_pool(name="oal", bufs=2) as oal, \
                     tc.tile_pool(name="p2t", bufs=2) as p2t, \
                     tc.tile_pool(name="pexp", bufs=2) as pexp, \
                     tc.tile_pool(name="pss", bufs=2, space="PSUM") as pss, \
                     tc.tile_pool(name="pso", bufs=2, space="PSUM") as pso:
                    wp_sb = [wpp.tile([128, C], BF16, name=f"wp{j}")
                             for j in range(6)]
                    for j in range(6):
                        nc.sync.dma_start(out=wp_sb[j][:], in_=wp_d[j])
                    qb = [qbp.tile([128, N], BF16, name=f"qb{par}")
                          for par in range(2)]
                    nc.vector.memset(qb[0][64:128, :], 0.0)
                    nc.vector.memset(qb[1][0:64, :], 0.0)

                    estate = {}

                    def emit_scores(b, h):
                        base = b * N
                        par = h % 2
                        p0 = par * 64
                        qsl = qk_sb[h // 2][p0:p0 + 64, base:base + N]
                        nc.scalar.activation(qb[par][p0:p0 + 64, :], qsl,
                                             AF.Identity)
                        exps = []
                        for i, (kt0, kr) in enumerate(KTILES):
                            ps_s = pss.tile([128, N], F32,
                                            name=f"ss{b}_{h}_{i}", tag="ps_s")
                            for (qc0, qcw) in QCH:
                                nc.tensor.matmul(
                                    ps_s[:kr, qc0:qc0 + qcw],
                                    qk_sb[6 + h // 2][:, base + kt0:
                                                      base + kt0 + kr],
                                    qb[par][:, qc0:qc0 + qcw],
                                    start=True, stop=True)
                            e = pexp.tile([128, N], BF16,
                                          name=f"e{b}_{h}_{i}", tag=f"e{i}")
                            nc.scalar.activation(e[:kr, :], ps_s[:kr, :],
                                                 AF.Exp)
                            exps.append((e, kr))
                        estate[(b, h)] = exps

                    oU65 = {}

                    def emit_av(b, h, den):
                        exps = estate.pop((b, h))
                        ps_o = pso.tile([65, N], F32, name=f"po{b}_{h}",
                                        tag="ps_o")
                        for (qc0, qcw) in QCH:
                            for i, (e, kr) in enumerate(exps):
                                nc.tensor.matmul(
                                    ps_o[:, qc0:qc0 + qcw],
                                    vbuf[(b, i)][:kr, h * 65:(h + 1) * 65],
                                    e[:kr, qc0:qc0 + qcw],
                                    start=(i == 0), stop=(i == 4))
                        # one copy releases the psum tile; row 64 carries the
                        # softmax denominator, gathered from SBUF below so the
                        # reciprocal chain never blocks psum recycling
                        oh = p2t.tile([65, N], BF16, name=f"oh{b}_{h}",
                                      tag=f"oh{h % 2}")
                        nc.vector.tensor_copy(oh[:], ps_o[:])
                        oU65[h % 4] = oh
                        dr = (h % 4) * 32
                        nc.vector.tensor_copy(den[dr:dr + 1, :],
                                              oh[64:65, :])

                    def emit_group_tail(b, g, den):
                        # den [97, N]: 4 heads' softmax denominators on
                        # quadrant partitions 0/32/64/96 (DVE reciprocal cost
                        # is flat in the partition count, so batch 4 heads per
                        # instruction). partition_broadcast only supports
                        # (src partition 0, dst base 0), so rows 32/64/96 are
                        # first copied down to partition 0.
                        rec = p2t.tile([97, N], BF16, name=f"rc{b}_{g}",
                                       tag="rec")
                        with nc.allow_low_precision(reason="softmax denom"):
                            nc.vector.reciprocal(rec[:], den[:])
                        srcs = {0: rec}
                        for r in (32, 64, 96):
                            t = p2t.tile([1, N], BF16, name=f"rx{b}_{g}_{r}",
                                         tag=f"rx{r}")
                            nc.vector.tensor_copy(t[:], rec[r:r + 1, :])
                            srcs[r] = t
                        for hh in range(4):
                            h = 4 * g + hh
                            bc = p2t.tile([64, N], BF16, name=f"bc{b}_{h}",
                                          tag=f"bc{hh}")
                            nc.gpsimd.partition_broadcast(
                                bc[:], srcs[hh * 32][0:1, :])
                            p0 = (h % 2) * 64
                            nc.vector.tensor_tensor(
                                o_tiles[b][h // 2][p0:p0 + 64, :],
                                oU65.pop(hh)[0:64, :], bc[:], ALU.mult)

                    def emit_proj(b):
                        base = b * N
                        for n in range(6):
                            pp = pss.tile([128, N], F32, name=f"pp{b}_{n}",
                                          tag="ps_s")
                            for (qc0, qcw) in QCH:
                                for j in range(6):
                                    nc.tensor.matmul(
                                        pp[:, qc0:qc0 + qcw],
                                        wp_sb[j][:, n * 128:(n + 1) * 128],
                                        o_tiles[b][j][:, qc0:qc0 + qcw],
                                        start=(j == 0), stop=(j == 5))
                            xr = p2t.tile([128, N], BF16, name=f"xr{b}_{n}",
                                          tag=f"xr{n % 2}")
                            nc.sync.dma_start(
                                out=xr[:],
                                in_=xT_d[n * 128:(n + 1) * 128,
                                         base:base + N])
                            nc.vector.scalar_tensor_tensor(
                                x2[n][:, base:base + N], pp[:],
                                pb[:, n:n + 1], xr[:], ALU.add, ALU.add)

                    o_tiles = {}
                    dens = {}
                    seq = [(b, h) for b in range(BPC) for h in range(H)]

                    def emit_av_tail(b, h):
                        if h % 4 == 0:
                            dens[b] = p2t.tile([97, N], F32, name=f"dn{b}_{h}",
                                               tag="den")
                            nc.vector.memset(dens[b][0:96, :], 1.0)
                        emit_av(b, h, dens[b])
                        if h % 4 == 3:
                            emit_group_tail(b, h // 4, dens.pop(b))

                    for idx, (b, h) in enumerate(seq):
                        if h == 0:
                            o_tiles[b] = [oal.tile([128, N], BF16,
                                                   name=f"oa{b}_{j}",
                                                   tag=f"oa{j}")
                                          for j in range(6)]
                        emit_scores(b, h)
                        if idx > 0:
                            emit_av_tail(*seq[idx - 1])
                        if idx > 2:
                            qb_, qh_ = seq[idx - 3]
                            if qh_ == H - 1:
                                emit_proj(qb_)
                    emit_av_tail(*seq[-1])
                    emit_proj(seq[-1][0])

            # ---------------- P4: LN2 + MLP ----------------
            with tc.tile_pool(name="w12", bufs=1) as w12, \
                 tc.tile_pool(name="p4x", bufs=2) as p4x, \
                 tc.tile_pool(name="p4s", bufs=2) as p4s, \
                 tc.tile_pool(name="h2p", bufs=2) as h2p, \
                 tc.tile_pool(name="p4t", bufs=2) as p4t, \
                 tc.tile_pool(name="pgl", bufs=1) as pgl, \
                 tc.tile_pool(name="ps4s", bufs=1, space="PSUM") as ps4s, \
                 tc.tile_pool(name="ps41", bufs=3, space="PSUM") as ps41, \
                 tc.tile_pool(name="ps42", bufs=2, space="PSUM") as ps42:
                w1_sb = [w12.tile([128, 2 * HID], F8, name=f"w1_{k}")
                         for k in range(3)]
                for k in range(3):
                    nc.sync.dma_start(out=w1_sb[k][:], in_=w1_d[k])
                w2_sb = [w12.tile([128, 2 * C], F8, name=f"w2_{k}")
                         for k in range(12)]
                h2c = {}

                def emit_stats2(ci):
                    c0, cw = CHUNKS[ci]
                    ps_sum = ps4s.tile([128, cw], F32, name=f"2ss_{c0}",
                                       tag="ps_sum2")
                    ps_ssq = ps4s.tile([128, cw], F32, name=f"2sq_{c0}",
                                       tag="ps_ssq2")
                    for k in range(6):
                        xq = p4x.tile([128, cw], BF16, name=f"2xq{k}_{c0}",
                                      tag=f"2xq{k}")
                        nc.vector.tensor_tensor(xq[:], x2[k][:, c0:c0 + cw],
                                                x2[k][:, c0:c0 + cw], ALU.mult)
                        nc.tensor.matmul(ps_sum[:], ones128[:],
                                         x2[k][:, c0:c0 + cw],
                                         start=(k == 0), stop=(k == 5))
                        nc.tensor.matmul(ps_ssq[:], ones128[:], xq[:],
                                         start=(k == 0), stop=(k == 5))
                    mu = p4s.tile([128, cw], F32, name=f"2mu_{c0}", tag="2mu")
                    nc.vector.tensor_scalar(mu[:], ps_sum[:], 1.0 / C, None,
                                            ALU.mult)
                    t1 = p4s.tile([128, cw], F32, name=f"2t1_{c0}", tag="2tt")
                    nc.vector.tensor_tensor(t1[:], ps_sum[:], mu[:], ALU.mult)
                    t2 = p4s.tile([128, cw], F32, name=f"2t2_{c0}", tag="2tt")
                    nc.vector.tensor_tensor(t2[:], ps_ssq[:], t1[:],
                                            ALU.subtract)
                    t3 = p4s.tile([128, cw], F32, name=f"2t3_{c0}", tag="2tt")
                    nc.vector.tensor_scalar(t3[:], t2[:], 1.0 / C, 1e-5,
                                            ALU.mult, ALU.add)
                    lnv = p4s.tile([128, cw], F32, name=f"2lv_{c0}", tag="2tt")
                    nc.scalar.activation(lnv[:], t3[:], AF.Ln)
                    rs = p4s.tile([128, cw], BF16, name=f"2rs_{c0}", tag="2rs")
                    nc.scalar.activation(rs[:], lnv[:], AF.Exp, scale=-0.5)
                    murs = p4s.tile([128, cw], BF16, name=f"2mr_{c0}",
                                    tag="2mr")
                    nc.vector.tensor_tensor(murs[:], mu[:], rs[:], ALU.mult)
                    hs = []
                    for i in range(3):
                        hp = h2p.tile([128, 2 * cw], F8, name=f"2h{i}_{c0}",
                                      tag=f"2h{i}")
                        hs.append(hp)
                    for k in range(6):
                        tmp = p4s.tile([128, cw], F32, name=f"2tp{k}_{c0}",
                                       tag="2tp")
                        nc.vector.tensor_tensor(tmp[:], x2[k][:, c0:c0 + cw],
                                                rs[:], ALU.mult)
                        dst = hs[k // 2][:, (k % 2) * cw:(k % 2) * cw + cw]
                        with nc.allow_low_precision(reason="fp8 mlp act"):
                            nc.vector.tensor_tensor(dst, tmp[:], murs[:],
                                                    ALU.subtract)
                    h2c[ci] = hs

                emit_stats2(0)
                for ci in range(5):
                    c0, cw = CHUNKS[ci]
                    hs = h2c.pop(ci)
                    gl = [pgl.tile([128, 2 * cw], F8, name=f"gl{j}_{c0}",
                                   tag=f"gl{j}") for j in range(12)]
                    for n1 in range(24):
                        p1p = ps41.tile([128, cw], F32, name=f"p41_{n1}_{c0}",
                                        tag="ps41")
                        for i in range(3):
                            w3 = w1_sb[i].rearrange(
                                "p (s n) -> p s n", s=2)[
                                :, :, n1 * 128:(n1 + 1) * 128]
                            h3 = hs[i].rearrange("p (s f) -> p s f", s=2)
                            nc.tensor.matmul(p1p[:], w3, h3,
                                             start=(i == 0), stop=(i == 2),
                                             perf_mode=DR)
                        gdst = gl[n1 // 2][:, (n1 % 2) * cw:(n1 % 2) * cw + cw]
                        with nc.allow_low_precision(reason="fp8 mlp act"):
                            nc.scalar.activation(gdst, p1p[:], AF.Gelu,
                                                 bias=b1a[:, n1:n1 + 1],
                                                 scale=1.0 / WS)
                    if ci == 0:
                        for k in range(12):
                            nc.sync.dma_start(out=w2_sb[k][:], in_=w2_d[k])
                    if ci + 1 < 5:
                        emit_stats2(ci + 1)
                    for n2 in range(6):
                        p2p = ps42.tile([128, cw], F32, name=f"p42_{n2}_{c0}",
                                        tag="ps42")
                        for j in range(12):
                            w3 = w2_sb[j].rearrange(
                                "p (s n) -> p s n", s=2)[
                                :, :, n2 * 128:(n2 + 1) * 128]
                            g3 = gl[j].rearrange("p (s f) -> p s f", s=2)
                            nc.tensor.matmul(p2p[:], w3, g3,
                                             start=(j == 0), stop=(j == 11),
                                             perf_mode=DR)
                        t2o = p4t.tile([128, cw], F32, name=f"t2o{n2}_{c0}",
                                       tag="t2o")
                        nc.scalar.activation(t2o[:], p2p[:], AF.Identity,
                                             bias=b2a[:, n2:n2 + 1],
                                             scale=1.0 / WS)
                        oo = p4t.tile([128, cw], F32, name=f"oo{n2}_{c0}",
                                      tag="oo")
                        nc.vector.tensor_tensor(oo[:], t2o[:],
                                                x2[n2][:, c0:c0 + cw], ALU.add)
                        nc.sync.dma_start(
                            out=outT_d[n2 * 128:(n2 + 1) * 128, c0:c0 + cw],
                            in_=oo[:])
    nc.compile()
    return nc


_CACHE = {}


def _prep_shared(inputs):
    f32 = np.float32
    bf = ml_dtypes.bfloat16
    WS_ = WS
    qkv_w = np.asarray(inputs["qkv_w"], f32)
    ln1_g = np.asarray(inputs["ln1_g"], f32)
    ln1_b = np.asarray(inputs["ln1_b"], f32)
    qkv_b = np.asarray(inputs["qkv_b"], f32)
    W = qkv_w * ln1_g[:, None]
    bq = ln1_b @ qkv_w + qkv_b
    W = W.copy()
    W[:, :C] *= 0.125
    bq = bq.copy()
    bq[:C] *= 0.125

    proj_w = np.asarray(inputs["proj_w"], f32)
    fc1_w = np.asarray(inputs["fc1_w"], f32)
    ln2_g = np.asarray(inputs["ln2_g"], f32)
    ln2_b = np.asarray(inputs["ln2_b"], f32)
    fc1_b = np.asarray(inputs["fc1_b"], f32)
    W1 = fc1_w * ln2_g[:, None]
    b1 = ln2_b @ fc1_w + fc1_b
    fc2_w = np.asarray(inputs["fc2_w"], f32)

    f8 = ml_dtypes.float8_e4m3
    w1s = (W1 * WS).astype(f8).reshape(3, 2, 128, HID)
    w1s = w1s.transpose(0, 2, 1, 3).reshape(3, 128, 2 * HID)
    w2s = (fc2_w * WS).astype(f8).reshape(12, 2, 128, C)
    w2s = w2s.transpose(0, 2, 1, 3).reshape(12, 128, 2 * C)
    return {
        "wqkv": np.ascontiguousarray(W.astype(bf)),
        "qkb": np.ascontiguousarray(bq[:2 * C].reshape(12, 128).T.astype(f32)),
        "vbb": np.ascontiguousarray(np.tile(bq[2 * C:], (128, 1)).astype(f32)),
        "wp": np.ascontiguousarray(proj_w.reshape(6, 128, C).astype(bf)),
        "pb": np.ascontiguousarray(
            np.asarray(inputs["proj_b"], f32).reshape(6, 128).T),
        "w1": np.ascontiguousarray(w1s),
        "b1a": np.ascontiguousarray(b1.reshape(24, 128).T.astype(f32)),
        "w2": np.ascontiguousarray(w2s),
        "b2a": np.ascontiguousarray(
            np.asarray(inputs["fc2_b"], f32).reshape(6, 128).T),
    }


def _make_in_maps(inputs):
    bf = ml_dtypes.bfloat16
    x = np.asarray(inputs["x"], np.float32)
    shared = _prep_shared(inputs)
    in_maps = []
    for c in range(NCORES):
        xT = np.ascontiguousarray(
            x[c * BPC:(c + 1) * BPC].reshape(T, C).T.astype(bf))
        m = {"xT": xT}
        m.update(shared)
        in_maps.append(m)
    return in_maps


def kernel(**inputs):
    if "nc" not in _CACHE:
        _CACHE["nc"] = _build_nc()
    nc = _CACHE["nc"]
    in_maps = _make_in_maps(inputs)
    res = run_bass_kernel_spmd(nc, in_maps, list(range(NCORES)))
    out = np.empty((B, N, C), np.float32)
    for c in range(NCORES):
        outT = res.results[c]["outT"]
        out[c * BPC:(c + 1) * BPC] = outT.T.reshape(BPC, N, C)
    return out
